# revision 6
# baseline (speedup 1.0000x reference)
"""Trainium2 Bass kernel for nn_DSCBR (gnn_message_passing), v3.

Strategy (8 NeuronCores, SPMD):
- Recursive 2-hop trimming: only rows observable in the final losses are
  computed (6.8M -> 3.6M edges).
- SpMM: dma_gather of source rows (int16-windowed) + selection-matrix
  matmuls. Selection matrices are HOST-BUILT fp8 (e4m3) with edge values
  and layer scale baked in (x32 to dodge subnormals; undone in the
  f32->bf16 cast on the Scalar engine).
- PSUM: one accumulation group per 2KB bank; 4 win-pairs share a bank
  (start zeroes the bank lazily); win-pairs stay resident across all
  source windows -> one [128,512] drain per bank per layer.
- Sharded AllGathers: compact ids are shard-major so f1/acc tables are
  all-gathered in slices as soon as their supergroups drain, overlapping
  the remaining gathers.
- L2 dest rows are a prefix of the L1 dest rows per core, so the
  epilogue's f1 rows are just a copy of the local L1 output in SBUF.
- Loss computed batch-sharded on compact tables + tiny AllReduce.
"""
import os
import sys
import types

sys.path.insert(0, "/opt/trn_rl_repo")

import numpy as np

import concourse.bass as bass
import concourse.bacc as bacc
import concourse.mybir as mybir
import concourse.tile as tile
from concourse.bass_utils import run_bass_kernel_spmd
from concourse.masks import make_identity

P = 128
NCORES = 8
SRC_WIN = 32768
GI_MAX = 3072          # idxs per dma_gather call (SWDGE ring: 256 descs; 3072 -> 193)
SGW = 16               # win-pairs per PSUM supergroup (4 banks)
D = 64
NU, NI, NB = 100000, 50000, 20000
BATCH = 2048
SEL_SCALE = 32.0
F32 = mybir.dt.float32
BF = mybir.dt.bfloat16
FP8 = mybir.dt.float8e4
I32 = mybir.dt.int32
I16 = mybir.dt.int16
AF = mybir.ActivationFunctionType
ALU = mybir.AluOpType


def cdiv(a, b):
    return -(-a // b)


# ---------------------------------------------------------------- host prep

def wrap_idx16(flat):
    return np.ascontiguousarray(np.tile(flat.reshape(-1, 16).T.astype(np.int16), (8, 1)))


def idx_cols_i32(flat):
    n = flat.shape[0]
    assert n % P == 0
    return np.ascontiguousarray(flat.reshape(-1, P).T.astype(np.int32))


class Compact:
    """Shard-major compact numbering of a row set dealt across cores.

    Slots per core are grouped into shards of `shard_slots`; global id =
    shard_off[s] + core*shard_rows[s] + (slot - s*shard_slots).
    """

    def __init__(self, pref_rows_percore, extra_rows, n_space, shard_slots):
        rows_pc = [list(x) for x in pref_rows_percore]
        for j, r in enumerate(extra_rows):
            rows_pc[j % NCORES].append(int(r))
        per = max(len(x) for x in rows_pc)
        self.R = cdiv(per, 2 * P) * (2 * P)
        self.shard_slots = shard_slots
        ns = cdiv(self.R, shard_slots)
        self.shard_rows = [min(shard_slots, self.R - s * shard_slots)
                           for s in range(ns)]
        self.shard_off = np.concatenate(
            [[0], np.cumsum([NCORES * r for r in self.shard_rows])]).astype(np.int64)
        self.V = int(self.shard_off[-1])
        self.core_rows = [np.asarray(x, np.int64) for x in rows_pc]
        self.comp_core = np.full(n_space, -1, np.int64)
        self.comp_slot = np.full(n_space, -1, np.int64)
        for c in range(NCORES):
            rr = self.core_rows[c]
            self.comp_core[rr] = c
            self.comp_slot[rr] = np.arange(len(rr))

    def gid_of_slot(self, core, slot):
        s = slot // self.shard_slots
        return (self.shard_off[s] + core * np.asarray(self.shard_rows)[s]
                + (slot - s * self.shard_slots))

    def gid(self, rows):
        core = self.comp_core[rows]
        slot = self.comp_slot[rows]
        assert (core >= 0).all()
        return self.gid_of_slot(core, slot)


def build_layer(dest_core, dest_slot, src_idx, val, R_out, V_src, scale):
    """Host-built per-core idx + fp8 sel streams and the emission program."""
    E = len(dest_core)
    core = dest_core
    wp = dest_slot // (2 * P)
    lrow = dest_slot % (2 * P)
    swin = src_idx // SRC_WIN
    sidx = src_idx % SRC_WIN
    nwp = R_out // (2 * P)
    nsrc = cdiv(V_src, SRC_WIN)
    nsg = cdiv(nwp, SGW)

    # Dedup: edges in the same (core, swin, wp) group with the same source
    # share one gather slot; their sel entries merge (sum on collision).
    order = np.lexsort((sidx, wp, swin, core))
    c_s, s_s, w_s, l_s, si_s, v_s = (core[order], swin[order], wp[order],
                                     lrow[order], sidx[order], val[order])
    gkey = (c_s * nsrc + s_s) * nwp + w_s
    new_g = np.concatenate([[True], gkey[1:] != gkey[:-1]])
    new_u = new_g | np.concatenate([[True], si_s[1:] != si_s[:-1]])
    uidx = np.cumsum(new_u) - 1
    grp_first_uidx = np.maximum.accumulate(np.where(new_g, uidx, 0))
    slot_in_grp = uidx - grp_first_uidx

    counts = np.zeros((NCORES, nsrc, nwp), np.int64)
    np.add.at(counts, (c_s[new_u], s_s[new_u], w_s[new_u]), 1)
    nchunks = cdiv(np.max(counts, axis=0), P)          # [nsrc, nwp]
    for w in range(nwp):
        if nchunks[:, w].sum() == 0:
            nchunks[0, w] = 1

    chunk_off = np.zeros((nsrc, nwp), np.int64)
    pos = 0
    for g in range(nsg):
        wps = range(g * SGW, min((g + 1) * SGW, nwp))
        for s in range(nsrc):
            for w in wps:
                chunk_off[s, w] = pos
                pos += nchunks[s, w]
    tch = pos

    epos = chunk_off[s_s, w_s] * P + slot_in_grp

    import ml_dtypes
    idx_streams = np.zeros((NCORES, tch * P), np.int16)
    sel8 = []
    for c in range(NCORES):
        m = c_s == c
        idx_streams[c, epos[m]] = si_s[m]
        selc = np.zeros((tch * P, 2 * P), np.float32)
        np.add.at(selc, (epos[m], l_s[m]), v_s[m] * scale)
        selc = selc.astype(ml_dtypes.float8_e4m3fn)
        sel8.append(np.ascontiguousarray(
            selc.reshape(tch, P, 2 * P).transpose(1, 0, 2).reshape(P, tch * 2 * P)))
    idx8 = [wrap_idx16(idx_streams[c]) for c in range(NCORES)]

    # Program: one PSUM accumulation group per 2KB bank (4 consecutive
    # win-pairs); start on the bank's first matmul of the sg, stop+drain on
    # its last.
    prog = []
    for g in range(nsg):
        wps = list(range(g * SGW, min((g + 1) * SGW, nwp)))
        flat = []
        for s in range(nsrc):
            for w in wps:
                take = int(nchunks[s, w])
                if take:
                    flat.append([s, w, take, False, False])
        bank_of = lambda w: (w - g * SGW) // 4
        for b in set(bank_of(it[1]) for it in flat):
            idxs = [i for i, it in enumerate(flat) if bank_of(it[1]) == b]
            flat[idxs[0]][3] = True
            flat[idxs[-1]][4] = True
        sws = []
        fi_pos = 0
        for s in range(nsrc):
            batches = []
            cur, cur_n = [], 0
            while fi_pos < len(flat) and flat[fi_pos][0] == s:
                _, w, take, st, sp = flat[fi_pos]
                fi_pos += 1
                while take > 0:
                    room = GI_MAX // P - cur_n
                    if room == 0:
                        batches.append(cur)
                        cur, cur_n = [], 0
                        room = GI_MAX // P
                    t = min(room, take)
                    cur.append((w, t, st, sp and t == take))
                    st = False
                    cur_n += t
                    take -= t
            if cur:
                batches.append(cur)
            if batches:
                sws.append((s, batches))
        prog.append((g, wps, sws))
    return dict(idx=idx8, sel=sel8, prog=prog, tch=tch, nwp=nwp, nsrc=nsrc,
                V_src=V_src)


def preprocess(inputs):
    u = np.asarray(inputs["users_feature"], np.float32)
    it = np.asarray(inputs["items_feature"], np.float32)
    b = np.asarray(inputs["bundles_feature"], np.float32)
    il_row = np.asarray(inputs["il_row"]).astype(np.int64)
    il_col = np.asarray(inputs["il_col"]).astype(np.int64)
    il_val = np.asarray(inputs["il_val"], np.float32)
    bl_row = np.asarray(inputs["bl_row"]).astype(np.int64)
    bl_col = np.asarray(inputs["bl_col"]).astype(np.int64)
    bl_val = np.asarray(inputs["bl_val"], np.float32)
    agg_row = np.asarray(inputs["agg_row"]).astype(np.int64)
    agg_col = np.asarray(inputs["agg_col"]).astype(np.int64)
    agg_val = np.asarray(inputs["agg_val"], np.float32)
    users = np.asarray(inputs["users"]).astype(np.int64)
    bundles = np.asarray(inputs["bundles"]).astype(np.int64)

    N1, N2 = NU + NI, NU + NB

    # ---- need sets (2-hop trimming)
    uu = np.unique(users)
    need_b = np.unique(bundles)
    bmask = np.zeros(NB, bool); bmask[need_b] = True
    agg_keep = bmask[agg_row]
    need_i = np.unique(agg_col[agg_keep])
    umask = np.zeros(NU, bool); umask[uu] = True
    imask = np.zeros(NI, bool); imask[need_i] = True

    m_il2 = np.concatenate([umask, imask])
    e_il2 = m_il2[il_row]
    need_f1_il = np.zeros(N1, bool)
    need_f1_il[np.unique(il_col[e_il2])] = True
    need_f1_il |= m_il2
    e_il1 = need_f1_il[il_row]

    m_bl2 = np.concatenate([umask, bmask])
    e_bl2 = m_bl2[bl_row]
    need_f1_bl = np.zeros(N2, bool)
    need_f1_bl[np.unique(bl_col[e_bl2])] = True
    need_f1_bl |= m_bl2
    e_bl1 = need_f1_bl[bl_row]

    # ---- compact maps: L2 sets first; their per-core rows become the slot
    # prefix of the L1 compaction (so f1 rows for the epilogue are local).
    SH1 = 2 * SGW * 2 * P      # 8192 slots/core per shard (2 sgs)
    SH2 = SGW * 2 * P          # 4096 (1 sg)
    # users first in the IL acc table so loss rows live in shard 0
    u_pref = [uu[c::NCORES] for c in range(NCORES)]
    m_il2_rest = m_il2.copy()
    m_il2_rest[uu] = False
    c_il2 = Compact(u_pref, np.nonzero(m_il2_rest)[0], N1, SH2)
    c_bl2 = Compact([[]] * NCORES, np.nonzero(m_bl2)[0], N2, SH2)
    rest_il1 = np.nonzero(need_f1_il & ~m_il2)[0]
    rest_bl1 = np.nonzero(need_f1_bl & ~m_bl2)[0]
    c_il1 = Compact(c_il2.core_rows, rest_il1, N1, SH1)
    c_bl1 = Compact(c_bl2.core_rows, rest_bl1, N2, SH1)
    c_ag = Compact([[]] * NCORES, need_b, NB, SH2)

    # ---- layer streams
    def layer(c_dst, erows, ecols_gid, evals, V_src, scale):
        return build_layer(c_dst.comp_core[erows], c_dst.comp_slot[erows],
                           ecols_gid, evals, c_dst.R, V_src, scale)

    il1 = layer(c_il1, il_row[e_il1], il_col[e_il1], il_val[e_il1], N1, 0.5)
    bl1 = layer(c_bl1, bl_row[e_bl1], bl_col[e_bl1], bl_val[e_bl1], N2, 0.5)
    il2 = layer(c_il2, il_row[e_il2], c_il1.gid(il_col[e_il2]), il_val[e_il2],
                c_il1.V, 1.0 / 3.0)
    bl2 = layer(c_bl2, bl_row[e_bl2], c_bl1.gid(bl_col[e_bl2]), bl_val[e_bl2],
                c_bl1.V, 1.0 / 3.0)
    agr, agc, agv = agg_row[agg_keep], agg_col[agg_keep], agg_val[agg_keep]
    ag = layer(c_ag, agr, c_il2.gid(NU + agc), agv, c_il2.V, 1.0)

    # ---- f0 tables and per-core compact-f0 epilogue inputs
    f0_il = np.concatenate([u, it], 0)
    f0_bl = np.concatenate([u, b], 0)

    def f0c(comp, R, f0):
        out = []
        for c in range(NCORES):
            rows = comp.core_rows[c]
            fc = np.zeros((R, D), np.float32)
            fc[:len(rows)] = f0[rows]
            out.append(fc)
        return out

    f0c_il = f0c(c_il2, c_il2.R, f0_il)
    f0c_bl = f0c(c_bl2, c_bl2.R, f0_bl)

    # ---- loss index maps
    loss = {}
    bsh = BATCH // NCORES
    for c in range(NCORES):
        sl = slice(c * bsh, (c + 1) * bsh)
        loss[c] = dict(
            u_il=idx_cols_i32(c_il2.gid(users[sl])),
            u_bl=idx_cols_i32(c_bl2.gid(users[sl])),
            b_il0=idx_cols_i32(c_ag.gid(bundles[sl, 0])),
            b_il1=idx_cols_i32(c_ag.gid(bundles[sl, 1])),
            b_bl0=idx_cols_i32(c_bl2.gid(bundles[sl, 0] + NU)),
            b_bl1=idx_cols_i32(c_bl2.gid(bundles[sl, 1] + NU)),
        )
    aug_u_bl = idx_cols_i32(c_bl2.gid(users))
    aug_b0_bl = idx_cols_i32(c_bl2.gid(bundles[:, 0] + NU))

    return dict(f0_il=f0_il, f0_bl=f0_bl,
                il1=il1, bl1=bl1, il2=il2, bl2=bl2, ag=ag,
                f0c_il=f0c_il, f0c_bl=f0c_bl,
                loss=loss, aug_u_bl=aug_u_bl, aug_b0_bl=aug_b0_bl,
                c_il1=c_il1, c_bl1=c_bl1, c_il2=c_il2, c_bl2=c_bl2, c_ag=c_ag)


# ---------------------------------------------------------------- bass build

class Ctx:
    pass


def emit_spmm(cx, name, layer, windows, idx_dram, sel_dram, out_sb, sg_hook=None):
    """One SpMM layer. windows: swin -> AP of the source rows.
    out_sb[:, w*64:(w+1)*64] receives dest window w. sg_hook(g) is called
    after supergroup g's drains are emitted."""
    nc = cx.nc
    chunk_pos = 0
    for (g, wps, sws) in layer["prog"]:
        banks = {}

        def bank_halves(w):
            b = (w - g * SGW) // 4
            if b not in banks:
                banks[b] = cx.psp.tile([P, 512], F32, space="PSUM",
                                       tag=f"bank{b}", name=f"bank{b}", bufs=1)
            t = banks[b]
            off = ((w - g * SGW) % 4) * 2 * D
            return t, b, t[:, off:off + D], t[:, off + D:off + 2 * D]

        for (s, batches) in sws:
            src_slice = windows[s]
            for batch in batches:
                nch = sum(it[1] for it in batch)
                gi = nch * P
                idx_t = cx.idxp.tile([P, GI_MAX // 16], I16, tag="gidx", name=f"{name}_gidx")
                nc.sync.dma_start(out=idx_t[:, :gi // 16],
                                  in_=idx_dram[:, chunk_pos * 8: chunk_pos * 8 + gi // 16])
                sel_t = cx.selp.tile([P, (GI_MAX // P) * 2 * P], FP8, tag="gsel",
                                     name=f"{name}_gsel")
                nc.sync.dma_start(
                    out=sel_t[:, :nch * 2 * P],
                    in_=sel_dram[:, chunk_pos * 2 * P: (chunk_pos + nch) * 2 * P])
                g_t = cx.gp.tile([P, (GI_MAX // P) * D], F32, tag="gg", name=f"{name}_gg")
                nc.gpsimd.dma_gather(
                    out_ap=g_t[:, :nch * D].rearrange("p (c d) -> p c d", c=nch),
                    in_ap=src_slice,
                    idxs_ap=idx_t[:, :gi // 16],
                    num_idxs=gi, num_idxs_reg=gi, elem_size=D,
                    single_packet=False)
                gbf = cx.gbp.tile([P, (GI_MAX // P) * D], BF, tag="gbf", name=f"{name}_gbf")
                nc.scalar.activation(gbf[:, :nch * D], g_t[:, :nch * D], AF.Copy,
                                     scale=1.0 / SEL_SCALE)
                bc = 0
                for (w, ncw, bst, bsp) in batch:
                    bank_t, b, pA, pB = bank_halves(w)
                    for k in range(ncw):
                        c = bc + k
                        st = (bst and k == 0)
                        sp = (bsp and k == ncw - 1)
                        nc.tensor.matmul(out=pA, lhsT=sel_t[:, c * 2 * P: c * 2 * P + P],
                                         rhs=gbf[:, c * D:(c + 1) * D], start=st, stop=False,
                                         skip_group_check=True)
                        nc.tensor.matmul(out=pB, lhsT=sel_t[:, c * 2 * P + P: (c + 1) * 2 * P],
                                         rhs=gbf[:, c * D:(c + 1) * D], start=False, stop=sp,
                                         skip_group_check=True)
                    if bsp:
                        w0 = g * SGW + 4 * b
                        nwin = min(4, len(wps) - 4 * b) * 2
                        nc.vector.tensor_copy(
                            out_sb[:, (2 * w0) * D:(2 * w0) * D + nwin * D],
                            bank_t[:, :nwin * D])
                    bc += ncw
                chunk_pos += nch
        if sg_hook is not None:
            sg_hook(g)
    assert chunk_pos == layer["tch"], f"{name}: {chunk_pos} != {layer['tch']}"


def emit_epilogue(cx, name, f0c_dram, f1c_sb, f2_sb, acc_sb, w0, w1):
    """acc[w0:w1 windows] = f0c + f1c/max(||f1c||,eps) + f2/max(||f2||,eps)."""
    nc = cx.nc
    nw = w1 - w0
    sl = slice(w0 * D, w1 * D)
    f0c = cx.ep.tile([P, nw * D], F32, tag="ep_f0", name=f"{name}_f0")
    nc.sync.dma_start(out=f0c[:].rearrange("p (w d) -> p w d", w=nw),
                      in_=f0c_dram[w0 * P:w1 * P, :].rearrange("(w p) d -> p w d", p=P))

    def normed(src, tag):
        sq = cx.ep.tile([P, nw * D], F32, tag="ep_sq", name=f"{name}_{tag}_sq")
        nc.vector.tensor_mul(sq[:], src, src)
        ss = cx.ep.tile([P, nw], F32, tag="ep_ss", name=f"{name}_{tag}_ss")
        nc.vector.reduce_sum(ss[:], sq[:].rearrange("p (w d) -> p w d", w=nw),
                             axis=mybir.AxisListType.X)
        sn = cx.ep.tile([P, nw], F32, tag="ep_sn", name=f"{name}_{tag}_sn")
        nc.scalar.activation(sn[:], ss[:], AF.Sqrt)
        nc.vector.tensor_scalar_max(sn[:], sn[:], 1e-12)
        rn = cx.ep.tile([P, nw], F32, tag="ep_rn", name=f"{name}_{tag}_rn")
        nc.vector.reciprocal(rn[:], sn[:])
        ct = cx.ep.tile([P, nw * D], F32, tag=f"ep_ct{tag}", name=f"{name}_{tag}_ct")
        nc.vector.tensor_mul(ct[:].rearrange("p (w d) -> p w d", w=nw),
                             src.rearrange("p (w d) -> p w d", w=nw),
                             rn[:].to_broadcast([P, nw, D]))
        return ct

    c1 = normed(f1c_sb[:, sl], "n1")
    c2 = normed(f2_sb[:, sl], "n2")
    nc.vector.tensor_add(acc_sb[:, sl], f0c[:], c1[:])
    nc.vector.tensor_add(acc_sb[:, sl], acc_sb[:, sl], c2[:])


def indirect_gather_rows(cx, out_sb, table_ap, idx_sb, ncols):
    nc = cx.nc
    for k in range(ncols):
        nc.gpsimd.indirect_dma_start(
            out=out_sb[:, k * D:(k + 1) * D], out_offset=None,
            in_=table_ap,
            in_offset=bass.IndirectOffsetOnAxis(ap=idx_sb[:, k:k + 1], axis=0))


def normalize_rows(cx, x_sb, ngroups, tag):
    nc = cx.nc
    sq = cx.lp.tile([P, ngroups * D], F32, tag=f"{tag}_sq")
    nc.vector.tensor_mul(sq[:], x_sb[:, :ngroups * D], x_sb[:, :ngroups * D])
    ss = cx.lp.tile([P, ngroups], F32, tag=f"{tag}_ss")
    nc.vector.reduce_sum(ss[:], sq[:].rearrange("p (w d) -> p w d", w=ngroups),
                         axis=mybir.AxisListType.X)
    sn = cx.lp.tile([P, ngroups], F32, tag=f"{tag}_sn")
    nc.scalar.activation(sn[:], ss[:], AF.Sqrt)
    nc.vector.tensor_scalar_max(sn[:], sn[:], 1e-12)
    rn = cx.lp.tile([P, ngroups], F32, tag=f"{tag}_rn")
    nc.vector.reciprocal(rn[:], sn[:])
    nc.vector.tensor_mul(
        x_sb[:, :ngroups * D].rearrange("p (w d) -> p w d", w=ngroups),
        x_sb[:, :ngroups * D].rearrange("p (w d) -> p w d", w=ngroups),
        rn[:].to_broadcast([P, ngroups, D]))


def rowdot(cx, a_sb, b_sb, out_sb, ngroups, tag):
    nc = cx.nc
    t = cx.lp.tile([P, ngroups * D], F32, tag=f"{tag}_t")
    nc.vector.tensor_mul(t[:], a_sb[:, :ngroups * D], b_sb[:, :ngroups * D])
    nc.vector.reduce_sum(out_sb[:, :ngroups], t[:].rearrange("p (w d) -> p w d", w=ngroups),
                         axis=mybir.AxisListType.X)


def transpose_groups(cx, src_sb, ngroups, tag):
    nc = cx.nc
    out = cx.lp.tile([P, ngroups * P], F32, tag=f"{tag}_T")
    for g in range(ngroups):
        pt = cx.psp.tile([P, 512], F32, space="PSUM", tag="trp", bufs=1)
        nc.tensor.transpose(out=pt[:D, :P], in_=src_sb[:, g * D:(g + 1) * D],
                            identity=cx.ident[:])
        nc.vector.tensor_copy(out[:D, g * P:(g + 1) * P], pt[:D, :P])
    return out


def build(pp):
    c_il1, c_bl1 = pp["c_il1"], pp["c_bl1"]
    c_il2, c_bl2, c_ag = pp["c_il2"], pp["c_bl2"], pp["c_ag"]
    nw_il1, nw_bl1 = c_il1.R // P, c_bl1.R // P
    nw_il2, nw_bl2, nw_ag = c_il2.R // P, c_bl2.R // P, c_ag.R // P

    nc = bacc.Bacc("TRN2", target_bir_lowering=False, debug=False, num_devices=NCORES)
    cx = Ctx()
    cx.nc = nc

    f0_il = nc.dram_tensor("f0_il", [NU + NI, D], F32, kind="ExternalInput")
    f0_bl = nc.dram_tensor("f0_bl", [NU + NB, D], F32, kind="ExternalInput")
    g_in = {}
    for nm in ("il1", "bl1", "il2", "bl2", "ag"):
        tch = pp[nm]["tch"]
        g_in[nm] = dict(
            idx=nc.dram_tensor(f"{nm}_idx", [P, tch * 8], I16, kind="ExternalInput"),
            sel=nc.dram_tensor(f"{nm}_sel", [P, tch * 2 * P], FP8, kind="ExternalInput"))
    f0c_il = nc.dram_tensor("f0c_il", [c_il2.R, D], F32, kind="ExternalInput")
    f0c_bl = nc.dram_tensor("f0c_bl", [c_bl2.R, D], F32, kind="ExternalInput")
    lidx = {k: nc.dram_tensor(f"loss_{k}", [P, v.shape[1]], I32, kind="ExternalInput")
            for k, v in pp["loss"][0].items()}
    aug_in = {k: nc.dram_tensor(k, [P, 16], I32, kind="ExternalInput")
              for k in ("aug_u_bl", "aug_b0_bl")}
    out_t = nc.dram_tensor("out", [1, 2], F32, kind="ExternalOutput")

    with tile.TileContext(nc) as tc:
        cx.tc = tc
        es = []
        def pool(name, bufs, **kw):
            p = tc.tile_pool(name=name, bufs=bufs, **kw)
            es.append(p)
            return p.__enter__()
        cx.psp = pool("psum", 1, space="PSUM")
        cx.dramp = pool("dram", 1, space="DRAM")
        cx.cp = pool("const", 1)
        cx.idxp = pool("gidx", 3)
        cx.selp = pool("gsel", 3)
        cx.gp = pool("gath", 2)
        cx.gbp = pool("gbf", 2)
        cx.ep = pool("epil", 1)
        cx.flp = pool("floc", 1)
        cx.lp = pool("loss", 1)

        flocal = cx.flp.tile([P, max(nw_il1, nw_bl1) * D], F32, tag="flocal",
                             name="flocal")
        acc_sb = cx.flp.tile([P, max(nw_il2, nw_bl2) * D], F32, tag="accsb",
                             name="accsb")
        f1c_il = cx.flp.tile([P, nw_il2 * D], F32, tag="f1c_il", name="f1c_il")
        f1c_bl = cx.flp.tile([P, nw_bl2 * D], F32, tag="f1c_bl", name="f1c_bl")

        def sharded_table(cm, nm):
            """DRAM tiles per shard + AG-in tile; returns (tiles, agin, windows).
            windows[s] = AP slice for source window s of the global gid space."""
            tiles = []
            for si, rows in enumerate(cm.shard_rows):
                tiles.append(cx.dramp.tile([NCORES * rows, D], F32,
                                           addr_space="Shared",
                                           tag=f"{nm}_t{si}", name=f"{nm}_t{si}"))
            agin = cx.dramp.tile([cm.R, D], F32, tag=f"{nm}_agin", name=f"{nm}_agin")
            windows = []
            off = 0
            for si, rows in enumerate(cm.shard_rows):
                gsz = NCORES * rows
                o = 0
                while o < gsz:
                    wr = min(SRC_WIN, gsz - o)
                    windows.append(tiles[si][o:o + wr, :])
                    o += wr
                off += gsz
            return tiles, agin, windows

        f1il_tiles, il1_agin, il2_windows = sharded_table(c_il1, "f1il")
        f1bl_tiles, bl1_agin, bl2_windows = sharded_table(c_bl1, "f1bl")
        ail_tiles, il2_agin, ag_windows = sharded_table(c_il2, "ail")
        abl_tiles, bl2_agin, _ = sharded_table(c_bl2, "abl")
        ilb_tiles, ag_agin, _ = sharded_table(c_ag, "ilb")

        def shard_ag(cm, flocal_sb, agin, tiles, si, extra_src=None):
            """DMA shard si's slots from flocal (or extra_src) and AllGather."""
            s0 = si * cm.shard_slots
            rows = cm.shard_rows[si]
            src = extra_src if extra_src is not None else flocal_sb
            nc.sync.dma_start(
                out=agin[s0:s0 + rows, :].rearrange("(w p) d -> p w d", p=P),
                in_=src[:, (s0 // P) * D:((s0 + rows) // P) * D]
                    .rearrange("p (w d) -> p w d", w=rows // P))
            nc.gpsimd.collective_compute(
                "AllGather", ALU.bypass, replica_groups=[list(range(NCORES))],
                ins=[agin[s0:s0 + rows, :].opt()], outs=[tiles[si][:].opt()])

        def last_sg_of_shard(cm, g, nsg):
            # shard si covers sgs [si*k, si*k+k) where k = shard_slots/sg_slots
            k = cm.shard_slots // (SGW * 2 * P)
            if g == nsg - 1:
                return len(cm.shard_rows) - 1
            if (g + 1) % k == 0:
                return g // k
            return None

        # ---- phase 1: L1 both graphs, sharded AGs inline
        nsg1 = len(pp["il1"]["prog"])

        def il1_hook(g):
            si = last_sg_of_shard(c_il1, g, nsg1)
            if si is not None:
                shard_ag(c_il1, flocal, il1_agin, f1il_tiles, si)

        emit_spmm(cx, "il1", pp["il1"],
                  [f0_il[s * SRC_WIN:min((s + 1) * SRC_WIN, NU + NI), :]
                   for s in range(pp["il1"]["nsrc"])],
                  g_in["il1"]["idx"], g_in["il1"]["sel"], flocal, il1_hook)
        nc.vector.tensor_copy(f1c_il[:], flocal[:, :nw_il2 * D])

        nsg2 = len(pp["bl1"]["prog"])

        def bl1_hook(g):
            si = last_sg_of_shard(c_bl1, g, nsg2)
            if si is not None:
                shard_ag(c_bl1, flocal, bl1_agin, f1bl_tiles, si)

        emit_spmm(cx, "bl1", pp["bl1"],
                  [f0_bl[s * SRC_WIN:min((s + 1) * SRC_WIN, NU + NB), :]
                   for s in range(pp["bl1"]["nsrc"])],
                  g_in["bl1"]["idx"], g_in["bl1"]["sel"], flocal, bl1_hook)
        nc.vector.tensor_copy(f1c_bl[:], flocal[:, :nw_bl2 * D])

        # ---- phase 2: L2 BL first (small; its acc AllGather leaves the
        # end-of-run collective chain and hides under il2's gathers)
        nsg4 = len(pp["bl2"]["prog"])

        def bl2_hook(g):
            si = last_sg_of_shard(c_bl2, g, nsg4)
            if si is not None:
                w0 = si * c_bl2.shard_slots // P
                w1 = w0 + c_bl2.shard_rows[si] // P
                emit_epilogue(cx, f"bl2s{si}", f0c_bl, f1c_bl, flocal, acc_sb,
                              w0, w1)
                shard_ag(c_bl2, acc_sb, bl2_agin, abl_tiles, si)

        emit_spmm(cx, "bl2", pp["bl2"], bl2_windows, g_in["bl2"]["idx"],
                  g_in["bl2"]["sel"], flocal, bl2_hook)

        # ---- phase 3: L2 IL with per-shard epilogue + AG
        nsg3 = len(pp["il2"]["prog"])

        def il2_hook(g):
            si = last_sg_of_shard(c_il2, g, nsg3)
            if si is not None:
                w0 = si * c_il2.shard_slots // P
                w1 = w0 + c_il2.shard_rows[si] // P
                emit_epilogue(cx, f"il2s{si}", f0c_il, f1c_il, flocal, acc_sb,
                              w0, w1)
                shard_ag(c_il2, acc_sb, il2_agin, ail_tiles, si)

        emit_spmm(cx, "il2", pp["il2"], il2_windows, g_in["il2"]["idx"],
                  g_in["il2"]["sel"], flocal, il2_hook)

        # ---- phase 4: agg spmm + AG
        nsg5 = len(pp["ag"]["prog"])

        def ag_hook(g):
            si = last_sg_of_shard(c_ag, g, nsg5)
            if si is not None:
                shard_ag(c_ag, flocal, ag_agin, ilb_tiles, si)

        emit_spmm(cx, "ag", pp["ag"], ag_windows, g_in["ag"]["idx"],
                  g_in["ag"]["sel"], flocal, ag_hook)

        # ---- phase 5: loss
        cx.ident = cx.cp.tile([P, P], F32)
        make_identity(nc, cx.ident[:])
        ones_col = cx.cp.tile([P, 1], F32)
        nc.vector.memset(ones_col[:], 1.0)
        assert len(abl_tiles) == 1 and len(ilb_tiles) == 1
        bsh = BATCH // NCORES
        ng = bsh // P
        lidx_sb = {}
        for k, t in lidx.items():
            s = cx.lp.tile([P, t.shape[1]], I32, tag=f"li_{k}")
            nc.sync.dma_start(out=s[:], in_=t[:])
            lidx_sb[k] = s
        for k, t in aug_in.items():
            s = cx.lp.tile([P, 16], I32, tag=f"li_{k}")
            nc.sync.dma_start(out=s[:], in_=t[:])
            lidx_sb[k] = s

        ail_ap = ail_tiles[0][:]
        abl_ap = abl_tiles[0][:]
        ilb_ap = ilb_tiles[0][:]

        def gather(tag, ap, idxk, ncols):
            sb = cx.lp.tile([P, ncols * D], F32, tag=tag)
            indirect_gather_rows(cx, sb, ap, lidx_sb[idxk], ncols)
            return sb

        pos_u_il = gather("pos_u_il", ail_ap, "u_il", ng)
        pos_u_bl = gather("pos_u_bl", abl_ap, "u_bl", ng)
        b_bl0 = gather("b_bl0", abl_ap, "b_bl0", ng)
        b_bl1 = gather("b_bl1", abl_ap, "b_bl1", ng)
        aug_u = gather("aug_u", abl_ap, "aug_u_bl", 16)
        aug_b = gather("aug_b", abl_ap, "aug_b0_bl", 16)

        # normalized copy of pos_u_il for c1 (BPR later needs the raw rows)
        pos_u_il_n = cx.lp.tile([P, ng * D], F32, tag="pos_u_il_n")
        nc.vector.tensor_copy(pos_u_il_n[:], pos_u_il[:, :ng * D])
        normalize_rows(cx, pos_u_il_n, ng, "npu")
        normalize_rows(cx, aug_u, 16, "nau")
        normalize_rows(cx, aug_b, 16, "nab")

        part = cx.lp.tile([P, 4], F32, tag="parts")
        nc.vector.memset(part[:], 0.0)

        def closs_partial(pos_my, aug_full, aug_my_cols, out_col):
            posT = transpose_groups(cx, pos_my, ng, "pT")
            augT = transpose_groups(cx, aug_full, 16, "aT")
            ps = cx.lp.tile([P, ng], F32, tag="ps")
            rowdot(cx, pos_my, aug_my_cols, ps, ng, f"psd{out_col}")
            lse = cx.lp.tile([P, ng], F32, tag="lse")
            for g in range(ng):
                ttl_ps = cx.psp.tile([P, 512], F32, space="PSUM", tag="ttl", bufs=1)
                ttl = cx.lp.tile([P, BATCH], F32, tag="ttl")
                for nb_ in range(BATCH // 512):
                    nc.tensor.matmul(
                        out=ttl_ps[:, :512],
                        lhsT=posT[:D, g * P:(g + 1) * P],
                        rhs=augT[:D, nb_ * 512:(nb_ + 1) * 512],
                        start=True, stop=True)
                    nc.vector.tensor_copy(ttl[:, nb_ * 512:(nb_ + 1) * 512], ttl_ps[:, :512])
                mx = cx.lp.tile([P, 1], F32, tag="mx")
                nc.vector.reduce_max(mx[:], ttl[:].rearrange("p (w d) -> p w d", w=1),
                                     axis=mybir.AxisListType.X)
                nmx = cx.lp.tile([P, 1], F32, tag="nmx")
                nc.vector.tensor_scalar_mul(nmx[:], mx[:], -4.0)
                ex = cx.lp.tile([P, BATCH], F32, tag="ex")
                se = cx.lp.tile([P, 1], F32, tag="se")
                nc.scalar.activation(ex[:], ttl[:], AF.Exp, bias=nmx[:, :1], scale=4.0,
                                     accum_out=se[:, :1])
                ln = cx.lp.tile([P, 1], F32, tag="ln")
                nc.scalar.activation(ln[:], se[:], AF.Ln)
                m4 = cx.lp.tile([P, 1], F32, tag="m4")
                nc.vector.tensor_scalar_mul(m4[:], mx[:], 4.0)
                nc.vector.tensor_add(lse[:, g:g + 1], ln[:], m4[:])
            t4 = cx.lp.tile([P, ng], F32, tag="t4")
            nc.vector.tensor_scalar_mul(t4[:], ps[:], 4.0)
            nc.vector.tensor_tensor(out=t4[:], in0=t4[:], in1=lse[:], op=ALU.subtract)
            nc.vector.reduce_sum(part[:, out_col:out_col + 1],
                                 t4[:].rearrange("p (w d) -> p w d", w=1),
                                 axis=mybir.AxisListType.X)

        aug_u_my = gather("aug_u_my", abl_ap, "u_bl", ng)
        normalize_rows(cx, aug_u_my, ng, "naum")
        aug_b_my = gather("aug_b_my", abl_ap, "b_bl0", ng)
        normalize_rows(cx, aug_b_my, ng, "nabm")
        closs_partial(pos_u_il_n, aug_u, aug_u_my, 1)

        b_il0 = gather("b_il0", ilb_ap, "b_il0", ng)
        b_il1 = gather("b_il1", ilb_ap, "b_il1", ng)
        pr0 = cx.lp.tile([P, ng], F32, tag="pr0")
        pr1 = cx.lp.tile([P, ng], F32, tag="pr1")
        tmp = cx.lp.tile([P, ng], F32, tag="prt")
        rowdot(cx, pos_u_il, b_il0, pr0, ng, "d0")
        rowdot(cx, pos_u_bl, b_bl0, tmp, ng, "d1")
        nc.vector.tensor_add(pr0[:], pr0[:], tmp[:])
        rowdot(cx, pos_u_il, b_il1, pr1, ng, "d2")
        rowdot(cx, pos_u_bl, b_bl1, tmp, ng, "d3")
        nc.vector.tensor_add(pr1[:], pr1[:], tmp[:])
        x = cx.lp.tile([P, ng], F32, tag="bprx")
        nc.vector.tensor_tensor(out=x[:], in0=pr1[:], in1=pr0[:], op=ALU.subtract)
        negx = cx.lp.tile([P, ng], F32, tag="bprnx")
        nc.vector.tensor_scalar_mul(negx[:], x[:], -1.0)
        nax = cx.lp.tile([P, ng], F32, tag="bprax")
        nc.vector.tensor_tensor(out=nax[:], in0=x[:], in1=negx[:], op=ALU.min)
        e = cx.lp.tile([P, ng], F32, tag="bpre")
        nc.scalar.activation(e[:], nax[:], AF.Exp)
        nc.vector.tensor_scalar_add(e[:], e[:], 1.0)
        l1p = cx.lp.tile([P, ng], F32, tag="bprl")
        nc.scalar.activation(l1p[:], e[:], AF.Ln)
        sp = cx.lp.tile([P, ng], F32, tag="bprsp")
        nc.vector.tensor_scalar_max(sp[:], x[:], 0.0)
        nc.vector.tensor_add(sp[:], sp[:], l1p[:])
        nc.vector.reduce_sum(part[:, 0:1], sp[:].rearrange("p (w d) -> p w d", w=1),
                             axis=mybir.AxisListType.X)

        my_pos_b = cx.lp.tile([P, ng * D], F32, tag="my_pb")
        nc.vector.tensor_copy(my_pos_b[:], b_il0[:, :ng * D])
        normalize_rows(cx, my_pos_b, ng, "npb")
        closs_partial(my_pos_b, aug_b, aug_b_my, 2)

        pp_ps = cx.psp.tile([P, 512], F32, space="PSUM", tag="ppps", bufs=1)
        nc.tensor.matmul(out=pp_ps[:1, :4], lhsT=ones_col[:], rhs=part[:],
                         start=True, stop=True)
        psum_sb = cx.lp.tile([1, 4], F32, tag="psums")
        nc.vector.tensor_copy(psum_sb[:], pp_ps[:1, :4])
        ar_in = cx.dramp.tile([1, 4], F32, tag="ar_in")
        ar_out = cx.dramp.tile([1, 4], F32, addr_space="Shared", tag="ar_out")
        nc.sync.dma_start(out=ar_in[:], in_=psum_sb[:])
        nc.gpsimd.collective_compute(
            "AllReduce", ALU.add, replica_groups=[list(range(NCORES))],
            ins=[ar_in[:].opt()], outs=[ar_out[:].opt()])
        fin = cx.lp.tile([1, 4], F32, tag="fin")
        nc.sync.dma_start(out=fin[:], in_=ar_out[:])
        res = cx.lp.tile([1, 2], F32, tag="res")
        nc.vector.tensor_scalar_mul(res[:, 0:1], fin[:, 0:1], 1.0 / BATCH)
        t = cx.lp.tile([1, 1], F32, tag="rt")
        nc.vector.tensor_add(t[:], fin[:, 1:2], fin[:, 2:3])
        nc.vector.tensor_scalar_mul(res[:, 1:2], t[:], -0.5 / BATCH)
        nc.sync.dma_start(out=out_t[:], in_=res[:])

        for p in reversed(es):
            p.__exit__(None, None, None)
    nc.compile()
    return nc


# ---------------------------------------------------------------- entry point

def _install_ntff_hook():
    if "antenv.axon_hooks" in sys.modules:
        return
    try:
        mod = types.ModuleType("antenv.axon_hooks")
        _hook = [None]
        mod.set_axon_ntff_profile_hook = lambda h: _hook.__setitem__(0, h)
        mod.get_axon_ntff_profile_hook = lambda: _hook[0]
        sys.modules["antenv.axon_hooks"] = mod
        import antenv
        antenv.axon_hooks = mod
        from trn_agent_boot.trn_boot import _ntff_profile_via_ctypes
        hook = _ntff_profile_via_ctypes("/opt/axon/libaxon_pjrt.so")
        if hook is not None:
            mod.set_axon_ntff_profile_hook(hook)
    except Exception:
        pass


def make_in_maps(pp):
    maps = []
    for c in range(NCORES):
        m = {
            "f0_il": pp["f0_il"], "f0_bl": pp["f0_bl"],
            "f0c_il": pp["f0c_il"][c], "f0c_bl": pp["f0c_bl"][c],
            "aug_u_bl": pp["aug_u_bl"], "aug_b0_bl": pp["aug_b0_bl"],
        }
        for nm in ("il1", "bl1", "il2", "bl2", "ag"):
            m[f"{nm}_idx"] = pp[nm]["idx"][c]
            m[f"{nm}_sel"] = pp[nm]["sel"][c]
        for k, v in pp["loss"][c].items():
            m[f"loss_{k}"] = v
        maps.append(m)
    return maps


_CACHE = {}


def kernel(**inputs) -> np.ndarray:
    _install_ntff_hook()
    pp = preprocess(inputs)
    key = "full"
    if key not in _CACHE:
        _CACHE[key] = build(pp)
    nc = _CACHE[key]
    in_maps = make_in_maps(pp)
    trace = bool(int(os.environ.get("DSCBR_TRACE", "0")))
    res = run_bass_kernel_spmd(nc, in_maps, core_ids=list(range(NCORES)), trace=trace)
    if trace and res.exec_time_ns:
        print(f"HW exec time: {res.exec_time_ns} ns")
    out = res.results[0]["out"].reshape(2).astype(np.float32)
    return out


# revision 8
# speedup vs baseline: 1.2406x; 1.2406x over previous
"""Trainium2 Bass kernel for nn_DSCBR (gnn_message_passing), v3.

Strategy (8 NeuronCores, SPMD):
- Recursive 2-hop trimming: only rows observable in the final losses are
  computed (6.8M -> 3.6M edges).
- SpMM: dma_gather of source rows (int16-windowed) + selection-matrix
  matmuls. Selection matrices are HOST-BUILT fp8 (e4m3) with edge values
  and layer scale baked in (x32 to dodge subnormals; undone in the
  f32->bf16 cast on the Scalar engine).
- PSUM: one accumulation group per 2KB bank; 4 win-pairs share a bank
  (start zeroes the bank lazily); win-pairs stay resident across all
  source windows -> one [128,512] drain per bank per layer.
- Sharded AllGathers: compact ids are shard-major so f1/acc tables are
  all-gathered in slices as soon as their supergroups drain, overlapping
  the remaining gathers.
- L2 dest rows are a prefix of the L1 dest rows per core, so the
  epilogue's f1 rows are just a copy of the local L1 output in SBUF.
- Loss computed batch-sharded on compact tables + tiny AllReduce.
"""
import os
import sys
import types

sys.path.insert(0, "/opt/trn_rl_repo")

import numpy as np

import concourse.bass as bass
import concourse.bacc as bacc
import concourse.mybir as mybir
import concourse.tile as tile
from concourse.bass_utils import run_bass_kernel_spmd
from concourse.masks import make_identity

P = 128
NCORES = 8
SRC_WIN = 32768
GI_MAX = 3072          # idxs per dma_gather call (SWDGE ring: 256 descs; 3072 -> 193)
SGW = 16               # win-pairs per PSUM supergroup (4 banks)
D = 64
NU, NI, NB = 100000, 50000, 20000
BATCH = 2048
SEL_SCALE = 32.0
F32 = mybir.dt.float32
BF = mybir.dt.bfloat16
FP8 = mybir.dt.float8e4
I32 = mybir.dt.int32
I16 = mybir.dt.int16
AF = mybir.ActivationFunctionType
ALU = mybir.AluOpType


def cdiv(a, b):
    return -(-a // b)


# ---------------------------------------------------------------- host prep

def wrap_idx16(flat):
    return np.ascontiguousarray(np.tile(flat.reshape(-1, 16).T.astype(np.int16), (8, 1)))


def idx_cols_i32(flat):
    n = flat.shape[0]
    assert n % P == 0
    return np.ascontiguousarray(flat.reshape(-1, P).T.astype(np.int32))


class Compact:
    """Shard-major compact numbering of a row set dealt across cores.

    Slots per core are grouped into shards of `shard_slots`; global id =
    shard_off[s] + core*shard_rows[s] + (slot - s*shard_slots).
    """

    def __init__(self, pref_rows_percore, extra_rows, n_space, shard_slots):
        rows_pc = [list(x) for x in pref_rows_percore]
        for j, r in enumerate(extra_rows):
            rows_pc[j % NCORES].append(int(r))
        per = max(len(x) for x in rows_pc)
        self.R = cdiv(per, 2 * P) * (2 * P)
        self.shard_slots = shard_slots
        ns = cdiv(self.R, shard_slots)
        self.shard_rows = [min(shard_slots, self.R - s * shard_slots)
                           for s in range(ns)]
        self.shard_off = np.concatenate(
            [[0], np.cumsum([NCORES * r for r in self.shard_rows])]).astype(np.int64)
        self.V = int(self.shard_off[-1])
        self.core_rows = [np.asarray(x, np.int64) for x in rows_pc]
        self.comp_core = np.full(n_space, -1, np.int64)
        self.comp_slot = np.full(n_space, -1, np.int64)
        for c in range(NCORES):
            rr = self.core_rows[c]
            self.comp_core[rr] = c
            self.comp_slot[rr] = np.arange(len(rr))

    def gid_of_slot(self, core, slot):
        s = slot // self.shard_slots
        return (self.shard_off[s] + core * np.asarray(self.shard_rows)[s]
                + (slot - s * self.shard_slots))

    def gid(self, rows):
        core = self.comp_core[rows]
        slot = self.comp_slot[rows]
        assert (core >= 0).all()
        return self.gid_of_slot(core, slot)


def build_layer(dest_core, dest_slot, src_idx, val, R_out, V_src, scale):
    """Host-built per-core idx + fp8 sel streams and the emission program."""
    E = len(dest_core)
    core = dest_core
    wp = dest_slot // (2 * P)
    lrow = dest_slot % (2 * P)
    swin = src_idx // SRC_WIN
    sidx = src_idx % SRC_WIN
    nwp = R_out // (2 * P)
    nsrc = cdiv(V_src, SRC_WIN)
    nsg = cdiv(nwp, SGW)

    # Dedup: edges in the same (core, swin, wp) group with the same source
    # share one gather slot; their sel entries merge (sum on collision).
    order = np.lexsort((sidx, wp, swin, core))
    c_s, s_s, w_s, l_s, si_s, v_s = (core[order], swin[order], wp[order],
                                     lrow[order], sidx[order], val[order])
    gkey = (c_s * nsrc + s_s) * nwp + w_s
    new_g = np.concatenate([[True], gkey[1:] != gkey[:-1]])
    new_u = new_g | np.concatenate([[True], si_s[1:] != si_s[:-1]])
    uidx = np.cumsum(new_u) - 1
    grp_first_uidx = np.maximum.accumulate(np.where(new_g, uidx, 0))
    slot_in_grp = uidx - grp_first_uidx

    counts = np.zeros((NCORES, nsrc, nwp), np.int64)
    np.add.at(counts, (c_s[new_u], s_s[new_u], w_s[new_u]), 1)
    nchunks = cdiv(np.max(counts, axis=0), P)          # [nsrc, nwp]
    for w in range(nwp):
        if nchunks[:, w].sum() == 0:
            nchunks[0, w] = 1

    chunk_off = np.zeros((nsrc, nwp), np.int64)
    pos = 0
    for g in range(nsg):
        wps = range(g * SGW, min((g + 1) * SGW, nwp))
        for s in range(nsrc):
            for w in wps:
                chunk_off[s, w] = pos
                pos += nchunks[s, w]
    tch = pos

    epos = chunk_off[s_s, w_s] * P + slot_in_grp

    import ml_dtypes
    idx_streams = np.zeros((NCORES, tch * P), np.int16)
    sel8 = []
    for c in range(NCORES):
        m = c_s == c
        idx_streams[c, epos[m]] = si_s[m]
        selc = np.zeros((tch * P, 2 * P), np.float32)
        np.add.at(selc, (epos[m], l_s[m]), v_s[m] * scale)
        selc = selc.astype(ml_dtypes.float8_e4m3fn)
        sel8.append(np.ascontiguousarray(
            selc.reshape(tch, P, 2 * P).transpose(1, 0, 2).reshape(P, tch * 2 * P)))
    idx8 = [wrap_idx16(idx_streams[c]) for c in range(NCORES)]

    # Program: one PSUM accumulation group per 2KB bank (4 consecutive
    # win-pairs); start on the bank's first matmul of the sg, stop+drain on
    # its last.
    prog = []
    for g in range(nsg):
        wps = list(range(g * SGW, min((g + 1) * SGW, nwp)))
        flat = []
        for s in range(nsrc):
            for w in wps:
                take = int(nchunks[s, w])
                if take:
                    flat.append([s, w, take, False, False])
        bank_of = lambda w: (w - g * SGW) // 4
        for b in set(bank_of(it[1]) for it in flat):
            idxs = [i for i, it in enumerate(flat) if bank_of(it[1]) == b]
            flat[idxs[0]][3] = True
            flat[idxs[-1]][4] = True
        sws = []
        fi_pos = 0
        for s in range(nsrc):
            batches = []
            cur, cur_n = [], 0
            while fi_pos < len(flat) and flat[fi_pos][0] == s:
                _, w, take, st, sp = flat[fi_pos]
                fi_pos += 1
                while take > 0:
                    room = GI_MAX // P - cur_n
                    if room == 0:
                        batches.append(cur)
                        cur, cur_n = [], 0
                        room = GI_MAX // P
                    t = min(room, take)
                    cur.append((w, t, st, sp and t == take))
                    st = False
                    cur_n += t
                    take -= t
            if cur:
                batches.append(cur)
            if batches:
                sws.append((s, batches))
        prog.append((g, wps, sws))
    return dict(idx=idx8, sel=sel8, prog=prog, tch=tch, nwp=nwp, nsrc=nsrc,
                V_src=V_src)


def preprocess(inputs):
    u = np.asarray(inputs["users_feature"], np.float32)
    it = np.asarray(inputs["items_feature"], np.float32)
    b = np.asarray(inputs["bundles_feature"], np.float32)
    il_row = np.asarray(inputs["il_row"]).astype(np.int64)
    il_col = np.asarray(inputs["il_col"]).astype(np.int64)
    il_val = np.asarray(inputs["il_val"], np.float32)
    bl_row = np.asarray(inputs["bl_row"]).astype(np.int64)
    bl_col = np.asarray(inputs["bl_col"]).astype(np.int64)
    bl_val = np.asarray(inputs["bl_val"], np.float32)
    agg_row = np.asarray(inputs["agg_row"]).astype(np.int64)
    agg_col = np.asarray(inputs["agg_col"]).astype(np.int64)
    agg_val = np.asarray(inputs["agg_val"], np.float32)
    users = np.asarray(inputs["users"]).astype(np.int64)
    bundles = np.asarray(inputs["bundles"]).astype(np.int64)

    N1, N2 = NU + NI, NU + NB

    # ---- need sets (2-hop trimming)
    uu = np.unique(users)
    need_b = np.unique(bundles)
    bmask = np.zeros(NB, bool); bmask[need_b] = True
    agg_keep = bmask[agg_row]
    need_i = np.unique(agg_col[agg_keep])
    umask = np.zeros(NU, bool); umask[uu] = True
    imask = np.zeros(NI, bool); imask[need_i] = True

    m_il2 = np.concatenate([umask, imask])
    e_il2 = m_il2[il_row]
    need_f1_il = np.zeros(N1, bool)
    need_f1_il[np.unique(il_col[e_il2])] = True
    need_f1_il |= m_il2
    e_il1 = need_f1_il[il_row]

    m_bl2 = np.concatenate([umask, bmask])
    e_bl2 = m_bl2[bl_row]
    need_f1_bl = np.zeros(N2, bool)
    need_f1_bl[np.unique(bl_col[e_bl2])] = True
    need_f1_bl |= m_bl2
    e_bl1 = need_f1_bl[bl_row]

    # ---- compact maps: L2 sets first; their per-core rows become the slot
    # prefix of the L1 compaction (so f1 rows for the epilogue are local).
    SH1 = 2 * SGW * 2 * P      # 8192 slots/core per shard (2 sgs)
    SH2 = SGW * 2 * P          # 4096 (1 sg)
    # users first in the IL acc table so loss rows live in shard 0
    u_pref = [uu[c::NCORES] for c in range(NCORES)]
    m_il2_rest = m_il2.copy()
    m_il2_rest[uu] = False
    c_il2 = Compact(u_pref, np.nonzero(m_il2_rest)[0], N1, SH2)
    c_bl2 = Compact([[]] * NCORES, np.nonzero(m_bl2)[0], N2, SH2)
    rest_il1 = np.nonzero(need_f1_il & ~m_il2)[0]
    rest_bl1 = np.nonzero(need_f1_bl & ~m_bl2)[0]
    c_il1 = Compact(c_il2.core_rows, rest_il1, N1, SH1)
    c_bl1 = Compact(c_bl2.core_rows, rest_bl1, N2, SH1)
    c_ag = Compact([[]] * NCORES, need_b, NB, SH2)

    # ---- layer streams
    def layer(c_dst, erows, ecols_gid, evals, V_src, scale):
        return build_layer(c_dst.comp_core[erows], c_dst.comp_slot[erows],
                           ecols_gid, evals, c_dst.R, V_src, scale)

    il1 = layer(c_il1, il_row[e_il1], il_col[e_il1], il_val[e_il1], N1, 0.5)
    bl1 = layer(c_bl1, bl_row[e_bl1], bl_col[e_bl1], bl_val[e_bl1], N2, 0.5)
    il2 = layer(c_il2, il_row[e_il2], c_il1.gid(il_col[e_il2]), il_val[e_il2],
                c_il1.V, 1.0 / 3.0)
    bl2 = layer(c_bl2, bl_row[e_bl2], c_bl1.gid(bl_col[e_bl2]), bl_val[e_bl2],
                c_bl1.V, 1.0 / 3.0)
    agr, agc, agv = agg_row[agg_keep], agg_col[agg_keep], agg_val[agg_keep]
    ag = layer(c_ag, agr, c_il2.gid(NU + agc), agv, c_il2.V, 1.0)

    # ---- f0 tables and per-core compact-f0 epilogue inputs
    f0_il = np.concatenate([u, it], 0)
    f0_bl = np.concatenate([u, b], 0)

    def f0c(comp, R, f0):
        out = []
        for c in range(NCORES):
            rows = comp.core_rows[c]
            fc = np.zeros((R, D), np.float32)
            fc[:len(rows)] = f0[rows]
            out.append(fc)
        return out

    f0c_il = f0c(c_il2, c_il2.R, f0_il)
    f0c_bl = f0c(c_bl2, c_bl2.R, f0_bl)

    # ---- loss index maps
    loss = {}
    bsh = BATCH // NCORES
    for c in range(NCORES):
        sl = slice(c * bsh, (c + 1) * bsh)
        loss[c] = dict(
            u_il=idx_cols_i32(c_il2.gid(users[sl])),
            u_bl=idx_cols_i32(c_bl2.gid(users[sl])),
            b_il0=idx_cols_i32(c_ag.gid(bundles[sl, 0])),
            b_il1=idx_cols_i32(c_ag.gid(bundles[sl, 1])),
            b_bl0=idx_cols_i32(c_bl2.gid(bundles[sl, 0] + NU)),
            b_bl1=idx_cols_i32(c_bl2.gid(bundles[sl, 1] + NU)),
        )
    aug_u_bl = idx_cols_i32(c_bl2.gid(users))
    aug_b0_bl = idx_cols_i32(c_bl2.gid(bundles[:, 0] + NU))

    return dict(f0_il=f0_il, f0_bl=f0_bl,
                il1=il1, bl1=bl1, il2=il2, bl2=bl2, ag=ag,
                f0c_il=f0c_il, f0c_bl=f0c_bl,
                loss=loss, aug_u_bl=aug_u_bl, aug_b0_bl=aug_b0_bl,
                c_il1=c_il1, c_bl1=c_bl1, c_il2=c_il2, c_bl2=c_bl2, c_ag=c_ag)


# ---------------------------------------------------------------- bass build

class Ctx:
    pass


def emit_spmm(cx, name, layer, windows, idx_dram, sel_dram, out_sb, sg_hook=None):
    """One SpMM layer. windows: swin -> AP of the source rows.
    out_sb[:, w*64:(w+1)*64] receives dest window w. sg_hook(g) is called
    after supergroup g's drains are emitted. Gather calls alternate between
    two SWDGE queues so one ring drains while the other fills."""
    nc = cx.nc
    chunk_pos = 0
    for (g, wps, sws) in layer["prog"]:
        banks = {}

        def bank_halves(w):
            b = (w - g * SGW) // 4
            if b not in banks:
                banks[b] = cx.psp.tile([P, 512], F32, space="PSUM",
                                       tag=f"bank{b}", name=f"bank{b}", bufs=1)
            t = banks[b]
            off = ((w - g * SGW) % 4) * 2 * D
            return t, b, t[:, off:off + D], t[:, off + D:off + 2 * D]

        for (s, batches) in sws:
            src_slice = windows[s]
            for batch in batches:
                nch = sum(it[1] for it in batch)
                gi = nch * P
                idx_t = cx.idxp.tile([P, GI_MAX // 16], I16, tag="gidx", name=f"{name}_gidx")
                nc.sync.dma_start(out=idx_t[:, :gi // 16],
                                  in_=idx_dram[:, chunk_pos * 8: chunk_pos * 8 + gi // 16])
                sel_t = cx.selp.tile([P, (GI_MAX // P) * 2 * P], FP8, tag="gsel",
                                     name=f"{name}_gsel")
                nc.sync.dma_start(
                    out=sel_t[:, :nch * 2 * P],
                    in_=sel_dram[:, chunk_pos * 2 * P: (chunk_pos + nch) * 2 * P])
                g_t = cx.gp.tile([P, (GI_MAX // P) * D], F32, tag="gg", name=f"{name}_gg")
                cx.gq = getattr(cx, "gq", 0) ^ 1
                nc.gpsimd.dma_gather(
                    out_ap=g_t[:, :nch * D].rearrange("p (c d) -> p c d", c=nch),
                    in_ap=src_slice,
                    idxs_ap=idx_t[:, :gi // 16],
                    num_idxs=gi, num_idxs_reg=gi, elem_size=D,
                    single_packet=False, queue_num=cx.gq)
                gbf = cx.gbp.tile([P, (GI_MAX // P) * D], BF, tag="gbf", name=f"{name}_gbf")
                nc.scalar.activation(gbf[:, :nch * D], g_t[:, :nch * D], AF.Copy,
                                     scale=1.0 / SEL_SCALE)
                bc = 0
                for (w, ncw, bst, bsp) in batch:
                    bank_t, b, pA, pB = bank_halves(w)
                    for k in range(ncw):
                        c = bc + k
                        st = (bst and k == 0)
                        sp = (bsp and k == ncw - 1)
                        nc.tensor.matmul(out=pA, lhsT=sel_t[:, c * 2 * P: c * 2 * P + P],
                                         rhs=gbf[:, c * D:(c + 1) * D], start=st, stop=False,
                                         skip_group_check=True)
                        nc.tensor.matmul(out=pB, lhsT=sel_t[:, c * 2 * P + P: (c + 1) * 2 * P],
                                         rhs=gbf[:, c * D:(c + 1) * D], start=False, stop=sp,
                                         skip_group_check=True)
                    if bsp:
                        w0 = g * SGW + 4 * b
                        nwin = min(4, len(wps) - 4 * b) * 2
                        nc.vector.tensor_copy(
                            out_sb[:, (2 * w0) * D:(2 * w0) * D + nwin * D],
                            bank_t[:, :nwin * D])
                    bc += ncw
                chunk_pos += nch
        if sg_hook is not None:
            sg_hook(g)
    assert chunk_pos == layer["tch"], f"{name}: {chunk_pos} != {layer['tch']}"


def emit_epilogue(cx, name, f0c_dram, f1c_sb, f2_sb, acc_sb, w0, w1):
    """acc[w0:w1 windows] = f0c + f1c/max(||f1c||,eps) + f2/max(||f2||,eps)."""
    nc = cx.nc
    nw = w1 - w0
    sl = slice(w0 * D, w1 * D)
    f0c = cx.ep.tile([P, nw * D], F32, tag="ep_f0", name=f"{name}_f0")
    nc.sync.dma_start(out=f0c[:].rearrange("p (w d) -> p w d", w=nw),
                      in_=f0c_dram[w0 * P:w1 * P, :].rearrange("(w p) d -> p w d", p=P))

    def normed(src, tag):
        sq = cx.ep.tile([P, nw * D], F32, tag="ep_sq", name=f"{name}_{tag}_sq")
        nc.vector.tensor_mul(sq[:], src, src)
        ss = cx.ep.tile([P, nw], F32, tag="ep_ss", name=f"{name}_{tag}_ss")
        nc.vector.reduce_sum(ss[:], sq[:].rearrange("p (w d) -> p w d", w=nw),
                             axis=mybir.AxisListType.X)
        sn = cx.ep.tile([P, nw], F32, tag="ep_sn", name=f"{name}_{tag}_sn")
        nc.scalar.activation(sn[:], ss[:], AF.Sqrt)
        nc.vector.tensor_scalar_max(sn[:], sn[:], 1e-12)
        rn = cx.ep.tile([P, nw], F32, tag="ep_rn", name=f"{name}_{tag}_rn")
        nc.vector.reciprocal(rn[:], sn[:])
        ct = cx.ep.tile([P, nw * D], F32, tag=f"ep_ct{tag}", name=f"{name}_{tag}_ct")
        nc.vector.tensor_mul(ct[:].rearrange("p (w d) -> p w d", w=nw),
                             src.rearrange("p (w d) -> p w d", w=nw),
                             rn[:].to_broadcast([P, nw, D]))
        return ct

    c1 = normed(f1c_sb[:, sl], "n1")
    c2 = normed(f2_sb[:, sl], "n2")
    nc.vector.tensor_add(acc_sb[:, sl], f0c[:], c1[:])
    nc.vector.tensor_add(acc_sb[:, sl], acc_sb[:, sl], c2[:])


def indirect_gather_rows(cx, out_sb, table_ap, idx_sb, ncols):
    nc = cx.nc
    for k in range(ncols):
        nc.gpsimd.indirect_dma_start(
            out=out_sb[:, k * D:(k + 1) * D], out_offset=None,
            in_=table_ap,
            in_offset=bass.IndirectOffsetOnAxis(ap=idx_sb[:, k:k + 1], axis=0))


def normalize_rows(cx, x_sb, ngroups, tag):
    nc = cx.nc
    sq = cx.lp.tile([P, ngroups * D], F32, tag=f"{tag}_sq")
    nc.vector.tensor_mul(sq[:], x_sb[:, :ngroups * D], x_sb[:, :ngroups * D])
    ss = cx.lp.tile([P, ngroups], F32, tag=f"{tag}_ss")
    nc.vector.reduce_sum(ss[:], sq[:].rearrange("p (w d) -> p w d", w=ngroups),
                         axis=mybir.AxisListType.X)
    sn = cx.lp.tile([P, ngroups], F32, tag=f"{tag}_sn")
    nc.scalar.activation(sn[:], ss[:], AF.Sqrt)
    nc.vector.tensor_scalar_max(sn[:], sn[:], 1e-12)
    rn = cx.lp.tile([P, ngroups], F32, tag=f"{tag}_rn")
    nc.vector.reciprocal(rn[:], sn[:])
    nc.vector.tensor_mul(
        x_sb[:, :ngroups * D].rearrange("p (w d) -> p w d", w=ngroups),
        x_sb[:, :ngroups * D].rearrange("p (w d) -> p w d", w=ngroups),
        rn[:].to_broadcast([P, ngroups, D]))


def rowdot(cx, a_sb, b_sb, out_sb, ngroups, tag):
    nc = cx.nc
    t = cx.lp.tile([P, ngroups * D], F32, tag=f"{tag}_t")
    nc.vector.tensor_mul(t[:], a_sb[:, :ngroups * D], b_sb[:, :ngroups * D])
    nc.vector.reduce_sum(out_sb[:, :ngroups], t[:].rearrange("p (w d) -> p w d", w=ngroups),
                         axis=mybir.AxisListType.X)


def transpose_groups(cx, src_sb, ngroups, tag):
    nc = cx.nc
    out = cx.lp.tile([P, ngroups * P], F32, tag=f"{tag}_T")
    for g in range(ngroups):
        pt = cx.psp.tile([P, 512], F32, space="PSUM", tag="trp", bufs=1)
        nc.tensor.transpose(out=pt[:D, :P], in_=src_sb[:, g * D:(g + 1) * D],
                            identity=cx.ident[:])
        nc.vector.tensor_copy(out[:D, g * P:(g + 1) * P], pt[:D, :P])
    return out


def build(pp):
    c_il1, c_bl1 = pp["c_il1"], pp["c_bl1"]
    c_il2, c_bl2, c_ag = pp["c_il2"], pp["c_bl2"], pp["c_ag"]
    nw_il1, nw_bl1 = c_il1.R // P, c_bl1.R // P
    nw_il2, nw_bl2, nw_ag = c_il2.R // P, c_bl2.R // P, c_ag.R // P

    nc = bacc.Bacc("TRN2", target_bir_lowering=False, debug=False, num_devices=NCORES,
                   num_swdge_queues=2, dynamic_dma_scratch_size=32768)
    cx = Ctx()
    cx.nc = nc

    f0_il = nc.dram_tensor("f0_il", [NU + NI, D], F32, kind="ExternalInput")
    f0_bl = nc.dram_tensor("f0_bl", [NU + NB, D], F32, kind="ExternalInput")
    g_in = {}
    for nm in ("il1", "bl1", "il2", "bl2", "ag"):
        tch = pp[nm]["tch"]
        g_in[nm] = dict(
            idx=nc.dram_tensor(f"{nm}_idx", [P, tch * 8], I16, kind="ExternalInput"),
            sel=nc.dram_tensor(f"{nm}_sel", [P, tch * 2 * P], FP8, kind="ExternalInput"))
    f0c_il = nc.dram_tensor("f0c_il", [c_il2.R, D], F32, kind="ExternalInput")
    f0c_bl = nc.dram_tensor("f0c_bl", [c_bl2.R, D], F32, kind="ExternalInput")
    lidx = {k: nc.dram_tensor(f"loss_{k}", [P, v.shape[1]], I32, kind="ExternalInput")
            for k, v in pp["loss"][0].items()}
    aug_in = {k: nc.dram_tensor(k, [P, 16], I32, kind="ExternalInput")
              for k in ("aug_u_bl", "aug_b0_bl")}
    out_t = nc.dram_tensor("out", [1, 2], F32, kind="ExternalOutput")

    with tile.TileContext(nc) as tc:
        cx.tc = tc
        es = []
        def pool(name, bufs, **kw):
            p = tc.tile_pool(name=name, bufs=bufs, **kw)
            es.append(p)
            return p.__enter__()
        cx.psp = pool("psum", 1, space="PSUM")
        cx.dramp = pool("dram", 1, space="DRAM")
        cx.cp = pool("const", 1)
        cx.idxp = pool("gidx", 3)
        cx.selp = pool("gsel", 3)
        cx.gp = pool("gath", 2)
        cx.gbp = pool("gbf", 2)
        cx.ep = pool("epil", 1)
        cx.flp = pool("floc", 1)
        cx.lp = pool("loss", 1)

        iota_i = cx.cp.tile([P, P], I32)
        nc.gpsimd.iota(iota_i[:], pattern=[[1, P]], base=0, channel_multiplier=0)
        cx.ident = cx.cp.tile([P, P], F32)
        make_identity(nc, cx.ident[:])
        ones_col = cx.cp.tile([P, 1], F32)
        nc.vector.memset(ones_col[:], 1.0)

        flocal = cx.flp.tile([P, max(nw_il1, nw_bl1) * D], F32, tag="flocal",
                             name="flocal")
        acc_sb = cx.flp.tile([P, max(nw_il2, nw_bl2) * D], F32, tag="accsb",
                             name="accsb")
        f1c_il = cx.flp.tile([P, nw_il2 * D], F32, tag="f1c_il", name="f1c_il")
        f1c_bl = cx.flp.tile([P, nw_bl2 * D], F32, tag="f1c_bl", name="f1c_bl")

        def sharded_table(cm, nm):
            """DRAM tiles per shard + AG-in tile; returns (tiles, agin, windows).
            windows[s] = AP slice for source window s of the global gid space."""
            tiles = []
            for si, rows in enumerate(cm.shard_rows):
                tiles.append(cx.dramp.tile([NCORES * rows, D], F32,
                                           addr_space="Shared",
                                           tag=f"{nm}_t{si}", name=f"{nm}_t{si}"))
            agin = cx.dramp.tile([cm.R, D], F32, tag=f"{nm}_agin", name=f"{nm}_agin")
            windows = []
            off = 0
            for si, rows in enumerate(cm.shard_rows):
                gsz = NCORES * rows
                o = 0
                while o < gsz:
                    wr = min(SRC_WIN, gsz - o)
                    windows.append(tiles[si][o:o + wr, :])
                    o += wr
                off += gsz
            return tiles, agin, windows

        f1il_tiles, il1_agin, il2_windows = sharded_table(c_il1, "f1il")
        f1bl_tiles, bl1_agin, bl2_windows = sharded_table(c_bl1, "f1bl")
        ail_tiles, il2_agin, ag_windows = sharded_table(c_il2, "ail")
        abl_tiles, bl2_agin, _ = sharded_table(c_bl2, "abl")
        ilb_tiles, ag_agin, _ = sharded_table(c_ag, "ilb")

        def shard_ag(cm, flocal_sb, agin, tiles, si, extra_src=None):
            """DMA shard si's slots from flocal (or extra_src) and AllGather."""
            s0 = si * cm.shard_slots
            rows = cm.shard_rows[si]
            src = extra_src if extra_src is not None else flocal_sb
            nc.sync.dma_start(
                out=agin[s0:s0 + rows, :].rearrange("(w p) d -> p w d", p=P),
                in_=src[:, (s0 // P) * D:((s0 + rows) // P) * D]
                    .rearrange("p (w d) -> p w d", w=rows // P))
            nc.gpsimd.collective_compute(
                "AllGather", ALU.bypass, replica_groups=[list(range(NCORES))],
                ins=[agin[s0:s0 + rows, :].opt()], outs=[tiles[si][:].opt()])

        def last_sg_of_shard(cm, g, nsg):
            # shard si covers sgs [si*k, si*k+k) where k = shard_slots/sg_slots
            k = cm.shard_slots // (SGW * 2 * P)
            if g == nsg - 1:
                return len(cm.shard_rows) - 1
            if (g + 1) % k == 0:
                return g // k
            return None

        # ---- phase 1: L1 both graphs, sharded AGs inline
        nsg1 = len(pp["il1"]["prog"])

        def il1_hook(g):
            si = last_sg_of_shard(c_il1, g, nsg1)
            if si is not None:
                shard_ag(c_il1, flocal, il1_agin, f1il_tiles, si)

        emit_spmm(cx, "il1", pp["il1"],
                  [f0_il[s * SRC_WIN:min((s + 1) * SRC_WIN, NU + NI), :]
                   for s in range(pp["il1"]["nsrc"])],
                  g_in["il1"]["idx"], g_in["il1"]["sel"], flocal, il1_hook)
        nc.vector.tensor_copy(f1c_il[:], flocal[:, :nw_il2 * D])

        nsg2 = len(pp["bl1"]["prog"])

        def bl1_hook(g):
            si = last_sg_of_shard(c_bl1, g, nsg2)
            if si is not None:
                shard_ag(c_bl1, flocal, bl1_agin, f1bl_tiles, si)

        emit_spmm(cx, "bl1", pp["bl1"],
                  [f0_bl[s * SRC_WIN:min((s + 1) * SRC_WIN, NU + NB), :]
                   for s in range(pp["bl1"]["nsrc"])],
                  g_in["bl1"]["idx"], g_in["bl1"]["sel"], flocal, bl1_hook)
        nc.vector.tensor_copy(f1c_bl[:], flocal[:, :nw_bl2 * D])

        # ---- phase 2: L2 IL with per-shard epilogue + AG
        nsg3 = len(pp["il2"]["prog"])

        def il2_hook(g):
            si = last_sg_of_shard(c_il2, g, nsg3)
            if si is not None:
                w0 = si * c_il2.shard_slots // P
                w1 = w0 + c_il2.shard_rows[si] // P
                emit_epilogue(cx, f"il2s{si}", f0c_il, f1c_il, flocal, acc_sb,
                              w0, w1)
                shard_ag(c_il2, acc_sb, il2_agin, ail_tiles, si)

        emit_spmm(cx, "il2", pp["il2"], il2_windows, g_in["il2"]["idx"],
                  g_in["il2"]["sel"], flocal, il2_hook)

        # ---- phase 3: L2 BL
        nsg4 = len(pp["bl2"]["prog"])

        def bl2_hook(g):
            si = last_sg_of_shard(c_bl2, g, nsg4)
            if si is not None:
                w0 = si * c_bl2.shard_slots // P
                w1 = w0 + c_bl2.shard_rows[si] // P
                emit_epilogue(cx, f"bl2s{si}", f0c_bl, f1c_bl, flocal, acc_sb,
                              w0, w1)
                shard_ag(c_bl2, acc_sb, bl2_agin, abl_tiles, si)

        emit_spmm(cx, "bl2", pp["bl2"], bl2_windows, g_in["bl2"]["idx"],
                  g_in["bl2"]["sel"], flocal, bl2_hook)

        # ---- phase 4: agg spmm + AG
        nsg5 = len(pp["ag"]["prog"])

        def ag_hook(g):
            si = last_sg_of_shard(c_ag, g, nsg5)
            if si is not None:
                shard_ag(c_ag, flocal, ag_agin, ilb_tiles, si)

        emit_spmm(cx, "ag", pp["ag"], ag_windows, g_in["ag"]["idx"],
                  g_in["ag"]["sel"], flocal, ag_hook)

        # ---- phase 5: loss
        assert len(abl_tiles) == 1 and len(ilb_tiles) == 1
        bsh = BATCH // NCORES
        ng = bsh // P
        lidx_sb = {}
        for k, t in lidx.items():
            s = cx.lp.tile([P, t.shape[1]], I32, tag=f"li_{k}")
            nc.sync.dma_start(out=s[:], in_=t[:])
            lidx_sb[k] = s
        for k, t in aug_in.items():
            s = cx.lp.tile([P, 16], I32, tag=f"li_{k}")
            nc.sync.dma_start(out=s[:], in_=t[:])
            lidx_sb[k] = s

        ail_ap = ail_tiles[0][:]
        abl_ap = abl_tiles[0][:]
        ilb_ap = ilb_tiles[0][:]

        def gather(tag, ap, idxk, ncols):
            sb = cx.lp.tile([P, ncols * D], F32, tag=tag)
            indirect_gather_rows(cx, sb, ap, lidx_sb[idxk], ncols)
            return sb

        pos_u_il = gather("pos_u_il", ail_ap, "u_il", ng)
        pos_u_bl = gather("pos_u_bl", abl_ap, "u_bl", ng)
        b_bl0 = gather("b_bl0", abl_ap, "b_bl0", ng)
        b_bl1 = gather("b_bl1", abl_ap, "b_bl1", ng)
        aug_u = gather("aug_u", abl_ap, "aug_u_bl", 16)
        aug_b = gather("aug_b", abl_ap, "aug_b0_bl", 16)

        # normalized copy of pos_u_il for c1 (BPR later needs the raw rows)
        pos_u_il_n = cx.lp.tile([P, ng * D], F32, tag="pos_u_il_n")
        nc.vector.tensor_copy(pos_u_il_n[:], pos_u_il[:, :ng * D])
        normalize_rows(cx, pos_u_il_n, ng, "npu")
        normalize_rows(cx, aug_u, 16, "nau")
        normalize_rows(cx, aug_b, 16, "nab")

        part = cx.lp.tile([P, 4], F32, tag="parts")
        nc.vector.memset(part[:], 0.0)

        def closs_partial(pos_my, aug_full, aug_my_cols, out_col):
            posT = transpose_groups(cx, pos_my, ng, "pT")
            augT = transpose_groups(cx, aug_full, 16, "aT")
            ps = cx.lp.tile([P, ng], F32, tag="ps")
            rowdot(cx, pos_my, aug_my_cols, ps, ng, f"psd{out_col}")
            lse = cx.lp.tile([P, ng], F32, tag="lse")
            for g in range(ng):
                ttl_ps = cx.psp.tile([P, 512], F32, space="PSUM", tag="ttl", bufs=1)
                ttl = cx.lp.tile([P, BATCH], F32, tag="ttl")
                for nb_ in range(BATCH // 512):
                    nc.tensor.matmul(
                        out=ttl_ps[:, :512],
                        lhsT=posT[:D, g * P:(g + 1) * P],
                        rhs=augT[:D, nb_ * 512:(nb_ + 1) * 512],
                        start=True, stop=True)
                    nc.vector.tensor_copy(ttl[:, nb_ * 512:(nb_ + 1) * 512], ttl_ps[:, :512])
                mx = cx.lp.tile([P, 1], F32, tag="mx")
                nc.vector.reduce_max(mx[:], ttl[:].rearrange("p (w d) -> p w d", w=1),
                                     axis=mybir.AxisListType.X)
                nmx = cx.lp.tile([P, 1], F32, tag="nmx")
                nc.vector.tensor_scalar_mul(nmx[:], mx[:], -4.0)
                ex = cx.lp.tile([P, BATCH], F32, tag="ex")
                se = cx.lp.tile([P, 1], F32, tag="se")
                nc.scalar.activation(ex[:], ttl[:], AF.Exp, bias=nmx[:, :1], scale=4.0,
                                     accum_out=se[:, :1])
                ln = cx.lp.tile([P, 1], F32, tag="ln")
                nc.scalar.activation(ln[:], se[:], AF.Ln)
                m4 = cx.lp.tile([P, 1], F32, tag="m4")
                nc.vector.tensor_scalar_mul(m4[:], mx[:], 4.0)
                nc.vector.tensor_add(lse[:, g:g + 1], ln[:], m4[:])
            t4 = cx.lp.tile([P, ng], F32, tag="t4")
            nc.vector.tensor_scalar_mul(t4[:], ps[:], 4.0)
            nc.vector.tensor_tensor(out=t4[:], in0=t4[:], in1=lse[:], op=ALU.subtract)
            nc.vector.reduce_sum(part[:, out_col:out_col + 1],
                                 t4[:].rearrange("p (w d) -> p w d", w=1),
                                 axis=mybir.AxisListType.X)

        aug_u_my = gather("aug_u_my", abl_ap, "u_bl", ng)
        normalize_rows(cx, aug_u_my, ng, "naum")
        aug_b_my = gather("aug_b_my", abl_ap, "b_bl0", ng)
        normalize_rows(cx, aug_b_my, ng, "nabm")
        closs_partial(pos_u_il_n, aug_u, aug_u_my, 1)

        b_il0 = gather("b_il0", ilb_ap, "b_il0", ng)
        b_il1 = gather("b_il1", ilb_ap, "b_il1", ng)
        pr0 = cx.lp.tile([P, ng], F32, tag="pr0")
        pr1 = cx.lp.tile([P, ng], F32, tag="pr1")
        tmp = cx.lp.tile([P, ng], F32, tag="prt")
        rowdot(cx, pos_u_il, b_il0, pr0, ng, "d0")
        rowdot(cx, pos_u_bl, b_bl0, tmp, ng, "d1")
        nc.vector.tensor_add(pr0[:], pr0[:], tmp[:])
        rowdot(cx, pos_u_il, b_il1, pr1, ng, "d2")
        rowdot(cx, pos_u_bl, b_bl1, tmp, ng, "d3")
        nc.vector.tensor_add(pr1[:], pr1[:], tmp[:])
        x = cx.lp.tile([P, ng], F32, tag="bprx")
        nc.vector.tensor_tensor(out=x[:], in0=pr1[:], in1=pr0[:], op=ALU.subtract)
        negx = cx.lp.tile([P, ng], F32, tag="bprnx")
        nc.vector.tensor_scalar_mul(negx[:], x[:], -1.0)
        nax = cx.lp.tile([P, ng], F32, tag="bprax")
        nc.vector.tensor_tensor(out=nax[:], in0=x[:], in1=negx[:], op=ALU.min)
        e = cx.lp.tile([P, ng], F32, tag="bpre")
        nc.scalar.activation(e[:], nax[:], AF.Exp)
        nc.vector.tensor_scalar_add(e[:], e[:], 1.0)
        l1p = cx.lp.tile([P, ng], F32, tag="bprl")
        nc.scalar.activation(l1p[:], e[:], AF.Ln)
        sp = cx.lp.tile([P, ng], F32, tag="bprsp")
        nc.vector.tensor_scalar_max(sp[:], x[:], 0.0)
        nc.vector.tensor_add(sp[:], sp[:], l1p[:])
        nc.vector.reduce_sum(part[:, 0:1], sp[:].rearrange("p (w d) -> p w d", w=1),
                             axis=mybir.AxisListType.X)

        my_pos_b = cx.lp.tile([P, ng * D], F32, tag="my_pb")
        nc.vector.tensor_copy(my_pos_b[:], b_il0[:, :ng * D])
        normalize_rows(cx, my_pos_b, ng, "npb")
        closs_partial(my_pos_b, aug_b, aug_b_my, 2)

        pp_ps = cx.psp.tile([P, 512], F32, space="PSUM", tag="ppps", bufs=1)
        nc.tensor.matmul(out=pp_ps[:1, :4], lhsT=ones_col[:], rhs=part[:],
                         start=True, stop=True)
        psum_sb = cx.lp.tile([1, 4], F32, tag="psums")
        nc.vector.tensor_copy(psum_sb[:], pp_ps[:1, :4])
        ar_in = cx.dramp.tile([1, 4], F32, tag="ar_in")
        ar_out = cx.dramp.tile([1, 4], F32, addr_space="Shared", tag="ar_out")
        nc.sync.dma_start(out=ar_in[:], in_=psum_sb[:])
        nc.gpsimd.collective_compute(
            "AllReduce", ALU.add, replica_groups=[list(range(NCORES))],
            ins=[ar_in[:].opt()], outs=[ar_out[:].opt()])
        fin = cx.lp.tile([1, 4], F32, tag="fin")
        nc.sync.dma_start(out=fin[:], in_=ar_out[:])
        res = cx.lp.tile([1, 2], F32, tag="res")
        nc.vector.tensor_scalar_mul(res[:, 0:1], fin[:, 0:1], 1.0 / BATCH)
        t = cx.lp.tile([1, 1], F32, tag="rt")
        nc.vector.tensor_add(t[:], fin[:, 1:2], fin[:, 2:3])
        nc.vector.tensor_scalar_mul(res[:, 1:2], t[:], -0.5 / BATCH)
        nc.sync.dma_start(out=out_t[:], in_=res[:])

        for p in reversed(es):
            p.__exit__(None, None, None)
    nc.compile()
    return nc


# ---------------------------------------------------------------- entry point

def _install_ntff_hook():
    if "antenv.axon_hooks" in sys.modules:
        return
    try:
        mod = types.ModuleType("antenv.axon_hooks")
        _hook = [None]
        mod.set_axon_ntff_profile_hook = lambda h: _hook.__setitem__(0, h)
        mod.get_axon_ntff_profile_hook = lambda: _hook[0]
        sys.modules["antenv.axon_hooks"] = mod
        import antenv
        antenv.axon_hooks = mod
        from trn_agent_boot.trn_boot import _ntff_profile_via_ctypes
        hook = _ntff_profile_via_ctypes("/opt/axon/libaxon_pjrt.so")
        if hook is not None:
            mod.set_axon_ntff_profile_hook(hook)
    except Exception:
        pass


def make_in_maps(pp):
    maps = []
    for c in range(NCORES):
        m = {
            "f0_il": pp["f0_il"], "f0_bl": pp["f0_bl"],
            "f0c_il": pp["f0c_il"][c], "f0c_bl": pp["f0c_bl"][c],
            "aug_u_bl": pp["aug_u_bl"], "aug_b0_bl": pp["aug_b0_bl"],
        }
        for nm in ("il1", "bl1", "il2", "bl2", "ag"):
            m[f"{nm}_idx"] = pp[nm]["idx"][c]
            m[f"{nm}_sel"] = pp[nm]["sel"][c]
        for k, v in pp["loss"][c].items():
            m[f"loss_{k}"] = v
        maps.append(m)
    return maps


_CACHE = {}


def kernel(**inputs) -> np.ndarray:
    _install_ntff_hook()
    pp = preprocess(inputs)
    key = "full"
    if key not in _CACHE:
        _CACHE[key] = build(pp)
    nc = _CACHE[key]
    in_maps = make_in_maps(pp)
    trace = bool(int(os.environ.get("DSCBR_TRACE", "0")))
    res = run_bass_kernel_spmd(nc, in_maps, core_ids=list(range(NCORES)), trace=trace)
    if trace and res.exec_time_ns:
        print(f"HW exec time: {res.exec_time_ns} ns")
    out = res.results[0]["out"].reshape(2).astype(np.float32)
    return out


# revision 9
# speedup vs baseline: 1.2444x; 1.0030x over previous
"""Trainium2 Bass kernel for nn_DSCBR (gnn_message_passing), v3.

Strategy (8 NeuronCores, SPMD):
- Recursive 2-hop trimming: only rows observable in the final losses are
  computed (6.8M -> 3.6M edges).
- SpMM: dma_gather of source rows (int16-windowed) + selection-matrix
  matmuls. Selection matrices are HOST-BUILT fp8 (e4m3) with edge values
  and layer scale baked in (x32 to dodge subnormals; undone in the
  f32->bf16 cast on the Scalar engine).
- PSUM: one accumulation group per 2KB bank; 4 win-pairs share a bank
  (start zeroes the bank lazily); win-pairs stay resident across all
  source windows -> one [128,512] drain per bank per layer.
- Sharded AllGathers: compact ids are shard-major so f1/acc tables are
  all-gathered in slices as soon as their supergroups drain, overlapping
  the remaining gathers.
- L2 dest rows are a prefix of the L1 dest rows per core, so the
  epilogue's f1 rows are just a copy of the local L1 output in SBUF.
- Loss computed batch-sharded on compact tables + tiny AllReduce.
"""
import os
import sys
import types

sys.path.insert(0, "/opt/trn_rl_repo")

import numpy as np

import concourse.bass as bass
import concourse.bacc as bacc
import concourse.mybir as mybir
import concourse.tile as tile
from concourse.bass_utils import run_bass_kernel_spmd
from concourse.masks import make_identity

P = 128
NCORES = 8
SRC_WIN = 32768
GI_MAX = 1920          # idxs/call: 121 descs fits 8KB rings (4-queue split)
SGW = 16               # win-pairs per PSUM supergroup (4 banks)
D = 64
NU, NI, NB = 100000, 50000, 20000
BATCH = 2048
SEL_SCALE = 32.0
F32 = mybir.dt.float32
BF = mybir.dt.bfloat16
FP8 = mybir.dt.float8e4
I32 = mybir.dt.int32
I16 = mybir.dt.int16
AF = mybir.ActivationFunctionType
ALU = mybir.AluOpType


def cdiv(a, b):
    return -(-a // b)


# ---------------------------------------------------------------- host prep

def wrap_idx16(flat):
    return np.ascontiguousarray(np.tile(flat.reshape(-1, 16).T.astype(np.int16), (8, 1)))


def idx_cols_i32(flat):
    n = flat.shape[0]
    assert n % P == 0
    return np.ascontiguousarray(flat.reshape(-1, P).T.astype(np.int32))


class Compact:
    """Shard-major compact numbering of a row set dealt across cores.

    Slots per core are grouped into shards of `shard_slots`; global id =
    shard_off[s] + core*shard_rows[s] + (slot - s*shard_slots).
    """

    def __init__(self, pref_rows_percore, extra_rows, n_space, shard_slots):
        rows_pc = [list(x) for x in pref_rows_percore]
        for j, r in enumerate(extra_rows):
            rows_pc[j % NCORES].append(int(r))
        per = max(len(x) for x in rows_pc)
        self.R = cdiv(per, 2 * P) * (2 * P)
        self.shard_slots = shard_slots
        ns = cdiv(self.R, shard_slots)
        self.shard_rows = [min(shard_slots, self.R - s * shard_slots)
                           for s in range(ns)]
        self.shard_off = np.concatenate(
            [[0], np.cumsum([NCORES * r for r in self.shard_rows])]).astype(np.int64)
        self.V = int(self.shard_off[-1])
        self.core_rows = [np.asarray(x, np.int64) for x in rows_pc]
        self.comp_core = np.full(n_space, -1, np.int64)
        self.comp_slot = np.full(n_space, -1, np.int64)
        for c in range(NCORES):
            rr = self.core_rows[c]
            self.comp_core[rr] = c
            self.comp_slot[rr] = np.arange(len(rr))

    def gid_of_slot(self, core, slot):
        s = slot // self.shard_slots
        return (self.shard_off[s] + core * np.asarray(self.shard_rows)[s]
                + (slot - s * self.shard_slots))

    def gid(self, rows):
        core = self.comp_core[rows]
        slot = self.comp_slot[rows]
        assert (core >= 0).all()
        return self.gid_of_slot(core, slot)


def build_layer(dest_core, dest_slot, src_idx, val, R_out, V_src, scale):
    """Host-built per-core idx + fp8 sel streams and the emission program."""
    E = len(dest_core)
    core = dest_core
    wp = dest_slot // (2 * P)
    lrow = dest_slot % (2 * P)
    swin = src_idx // SRC_WIN
    sidx = src_idx % SRC_WIN
    nwp = R_out // (2 * P)
    nsrc = cdiv(V_src, SRC_WIN)
    nsg = cdiv(nwp, SGW)

    # Dedup: edges in the same (core, swin, wp) group with the same source
    # share one gather slot; their sel entries merge (sum on collision).
    order = np.lexsort((sidx, wp, swin, core))
    c_s, s_s, w_s, l_s, si_s, v_s = (core[order], swin[order], wp[order],
                                     lrow[order], sidx[order], val[order])
    gkey = (c_s * nsrc + s_s) * nwp + w_s
    new_g = np.concatenate([[True], gkey[1:] != gkey[:-1]])
    new_u = new_g | np.concatenate([[True], si_s[1:] != si_s[:-1]])
    uidx = np.cumsum(new_u) - 1
    grp_first_uidx = np.maximum.accumulate(np.where(new_g, uidx, 0))
    slot_in_grp = uidx - grp_first_uidx

    counts = np.zeros((NCORES, nsrc, nwp), np.int64)
    np.add.at(counts, (c_s[new_u], s_s[new_u], w_s[new_u]), 1)
    nchunks = cdiv(np.max(counts, axis=0), P)          # [nsrc, nwp]
    for w in range(nwp):
        if nchunks[:, w].sum() == 0:
            nchunks[0, w] = 1

    chunk_off = np.zeros((nsrc, nwp), np.int64)
    pos = 0
    for g in range(nsg):
        wps = range(g * SGW, min((g + 1) * SGW, nwp))
        for s in range(nsrc):
            for w in wps:
                chunk_off[s, w] = pos
                pos += nchunks[s, w]
    tch = pos

    epos = chunk_off[s_s, w_s] * P + slot_in_grp

    import ml_dtypes
    idx_streams = np.zeros((NCORES, tch * P), np.int16)
    sel8 = []
    for c in range(NCORES):
        m = c_s == c
        idx_streams[c, epos[m]] = si_s[m]
        selc = np.zeros((tch * P, 2 * P), np.float32)
        np.add.at(selc, (epos[m], l_s[m]), v_s[m] * scale)
        selc = selc.astype(ml_dtypes.float8_e4m3fn)
        sel8.append(np.ascontiguousarray(
            selc.reshape(tch, P, 2 * P).transpose(1, 0, 2).reshape(P, tch * 2 * P)))
    idx8 = [wrap_idx16(idx_streams[c]) for c in range(NCORES)]

    # Program: one PSUM accumulation group per 2KB bank (4 consecutive
    # win-pairs); start on the bank's first matmul of the sg, stop+drain on
    # its last.
    prog = []
    for g in range(nsg):
        wps = list(range(g * SGW, min((g + 1) * SGW, nwp)))
        flat = []
        for s in range(nsrc):
            for w in wps:
                take = int(nchunks[s, w])
                if take:
                    flat.append([s, w, take, False, False])
        bank_of = lambda w: (w - g * SGW) // 4
        for b in set(bank_of(it[1]) for it in flat):
            idxs = [i for i, it in enumerate(flat) if bank_of(it[1]) == b]
            flat[idxs[0]][3] = True
            flat[idxs[-1]][4] = True
        sws = []
        fi_pos = 0
        for s in range(nsrc):
            batches = []
            cur, cur_n = [], 0
            while fi_pos < len(flat) and flat[fi_pos][0] == s:
                _, w, take, st, sp = flat[fi_pos]
                fi_pos += 1
                while take > 0:
                    room = GI_MAX // P - cur_n
                    if room == 0:
                        batches.append(cur)
                        cur, cur_n = [], 0
                        room = GI_MAX // P
                    t = min(room, take)
                    cur.append((w, t, st, sp and t == take))
                    st = False
                    cur_n += t
                    take -= t
            if cur:
                batches.append(cur)
            if batches:
                sws.append((s, batches))
        prog.append((g, wps, sws))
    return dict(idx=idx8, sel=sel8, prog=prog, tch=tch, nwp=nwp, nsrc=nsrc,
                V_src=V_src)


def preprocess(inputs):
    u = np.asarray(inputs["users_feature"], np.float32)
    it = np.asarray(inputs["items_feature"], np.float32)
    b = np.asarray(inputs["bundles_feature"], np.float32)
    il_row = np.asarray(inputs["il_row"]).astype(np.int64)
    il_col = np.asarray(inputs["il_col"]).astype(np.int64)
    il_val = np.asarray(inputs["il_val"], np.float32)
    bl_row = np.asarray(inputs["bl_row"]).astype(np.int64)
    bl_col = np.asarray(inputs["bl_col"]).astype(np.int64)
    bl_val = np.asarray(inputs["bl_val"], np.float32)
    agg_row = np.asarray(inputs["agg_row"]).astype(np.int64)
    agg_col = np.asarray(inputs["agg_col"]).astype(np.int64)
    agg_val = np.asarray(inputs["agg_val"], np.float32)
    users = np.asarray(inputs["users"]).astype(np.int64)
    bundles = np.asarray(inputs["bundles"]).astype(np.int64)

    N1, N2 = NU + NI, NU + NB

    # ---- need sets (2-hop trimming)
    uu = np.unique(users)
    need_b = np.unique(bundles)
    bmask = np.zeros(NB, bool); bmask[need_b] = True
    agg_keep = bmask[agg_row]
    need_i = np.unique(agg_col[agg_keep])
    umask = np.zeros(NU, bool); umask[uu] = True
    imask = np.zeros(NI, bool); imask[need_i] = True

    m_il2 = np.concatenate([umask, imask])
    e_il2 = m_il2[il_row]
    need_f1_il = np.zeros(N1, bool)
    need_f1_il[np.unique(il_col[e_il2])] = True
    need_f1_il |= m_il2
    e_il1 = need_f1_il[il_row]

    m_bl2 = np.concatenate([umask, bmask])
    e_bl2 = m_bl2[bl_row]
    need_f1_bl = np.zeros(N2, bool)
    need_f1_bl[np.unique(bl_col[e_bl2])] = True
    need_f1_bl |= m_bl2
    e_bl1 = need_f1_bl[bl_row]

    # ---- compact maps: L2 sets first; their per-core rows become the slot
    # prefix of the L1 compaction (so f1 rows for the epilogue are local).
    SH1 = 2 * SGW * 2 * P      # 8192 slots/core per shard (2 sgs)
    SH2 = SGW * 2 * P          # 4096 (1 sg)
    # users first in the IL acc table so loss rows live in shard 0
    u_pref = [uu[c::NCORES] for c in range(NCORES)]
    m_il2_rest = m_il2.copy()
    m_il2_rest[uu] = False
    c_il2 = Compact(u_pref, np.nonzero(m_il2_rest)[0], N1, SH2)
    c_bl2 = Compact([[]] * NCORES, np.nonzero(m_bl2)[0], N2, SH2)
    rest_il1 = np.nonzero(need_f1_il & ~m_il2)[0]
    rest_bl1 = np.nonzero(need_f1_bl & ~m_bl2)[0]
    c_il1 = Compact(c_il2.core_rows, rest_il1, N1, SH1)
    c_bl1 = Compact(c_bl2.core_rows, rest_bl1, N2, SH1)
    c_ag = Compact([[]] * NCORES, need_b, NB, SH2)

    # ---- layer streams
    def layer(c_dst, erows, ecols_gid, evals, V_src, scale):
        return build_layer(c_dst.comp_core[erows], c_dst.comp_slot[erows],
                           ecols_gid, evals, c_dst.R, V_src, scale)

    il1 = layer(c_il1, il_row[e_il1], il_col[e_il1], il_val[e_il1], N1, 0.5)
    bl1 = layer(c_bl1, bl_row[e_bl1], bl_col[e_bl1], bl_val[e_bl1], N2, 0.5)
    il2 = layer(c_il2, il_row[e_il2], c_il1.gid(il_col[e_il2]), il_val[e_il2],
                c_il1.V, 1.0 / 3.0)
    bl2 = layer(c_bl2, bl_row[e_bl2], c_bl1.gid(bl_col[e_bl2]), bl_val[e_bl2],
                c_bl1.V, 1.0 / 3.0)
    agr, agc, agv = agg_row[agg_keep], agg_col[agg_keep], agg_val[agg_keep]
    ag = layer(c_ag, agr, c_il2.gid(NU + agc), agv, c_il2.V, 1.0)

    # ---- f0 tables and per-core compact-f0 epilogue inputs
    f0_il = np.concatenate([u, it], 0)
    f0_bl = np.concatenate([u, b], 0)

    def f0c(comp, R, f0):
        out = []
        for c in range(NCORES):
            rows = comp.core_rows[c]
            fc = np.zeros((R, D), np.float32)
            fc[:len(rows)] = f0[rows]
            out.append(fc)
        return out

    f0c_il = f0c(c_il2, c_il2.R, f0_il)
    f0c_bl = f0c(c_bl2, c_bl2.R, f0_bl)

    # ---- loss index maps
    loss = {}
    bsh = BATCH // NCORES
    for c in range(NCORES):
        sl = slice(c * bsh, (c + 1) * bsh)
        loss[c] = dict(
            u_il=idx_cols_i32(c_il2.gid(users[sl])),
            u_bl=idx_cols_i32(c_bl2.gid(users[sl])),
            b_il0=idx_cols_i32(c_ag.gid(bundles[sl, 0])),
            b_il1=idx_cols_i32(c_ag.gid(bundles[sl, 1])),
            b_bl0=idx_cols_i32(c_bl2.gid(bundles[sl, 0] + NU)),
            b_bl1=idx_cols_i32(c_bl2.gid(bundles[sl, 1] + NU)),
        )
    aug_u_bl = idx_cols_i32(c_bl2.gid(users))
    aug_b0_bl = idx_cols_i32(c_bl2.gid(bundles[:, 0] + NU))

    return dict(f0_il=f0_il, f0_bl=f0_bl,
                il1=il1, bl1=bl1, il2=il2, bl2=bl2, ag=ag,
                f0c_il=f0c_il, f0c_bl=f0c_bl,
                loss=loss, aug_u_bl=aug_u_bl, aug_b0_bl=aug_b0_bl,
                c_il1=c_il1, c_bl1=c_bl1, c_il2=c_il2, c_bl2=c_bl2, c_ag=c_ag)


# ---------------------------------------------------------------- bass build

class Ctx:
    pass


def emit_spmm(cx, name, layer, windows, idx_dram, sel_dram, out_sb, sg_hook=None):
    """One SpMM layer. windows: swin -> AP of the source rows.
    out_sb[:, w*64:(w+1)*64] receives dest window w. sg_hook(g) is called
    after supergroup g's drains are emitted. Gather calls alternate between
    two SWDGE queues so one ring drains while the other fills."""
    nc = cx.nc
    chunk_pos = 0
    for (g, wps, sws) in layer["prog"]:
        banks = {}

        def bank_halves(w):
            b = (w - g * SGW) // 4
            if b not in banks:
                banks[b] = cx.psp.tile([P, 512], F32, space="PSUM",
                                       tag=f"bank{b}", name=f"bank{b}", bufs=1)
            t = banks[b]
            off = ((w - g * SGW) % 4) * 2 * D
            return t, b, t[:, off:off + D], t[:, off + D:off + 2 * D]

        for (s, batches) in sws:
            src_slice = windows[s]
            for batch in batches:
                nch = sum(it[1] for it in batch)
                gi = nch * P
                idx_t = cx.idxp.tile([P, GI_MAX // 16], I16, tag="gidx", name=f"{name}_gidx")
                nc.sync.dma_start(out=idx_t[:, :gi // 16],
                                  in_=idx_dram[:, chunk_pos * 8: chunk_pos * 8 + gi // 16])
                sel_t = cx.selp.tile([P, (GI_MAX // P) * 2 * P], FP8, tag="gsel",
                                     name=f"{name}_gsel")
                nc.sync.dma_start(
                    out=sel_t[:, :nch * 2 * P],
                    in_=sel_dram[:, chunk_pos * 2 * P: (chunk_pos + nch) * 2 * P])
                g_t = cx.gp.tile([P, (GI_MAX // P) * D], F32, tag="gg", name=f"{name}_gg")
                cx.gq = (getattr(cx, "gq", 3) + 1) % 4
                nc.gpsimd.dma_gather(
                    out_ap=g_t[:, :nch * D].rearrange("p (c d) -> p c d", c=nch),
                    in_ap=src_slice,
                    idxs_ap=idx_t[:, :gi // 16],
                    num_idxs=gi, num_idxs_reg=gi, elem_size=D,
                    single_packet=False, queue_num=cx.gq)
                gbf = cx.gbp.tile([P, (GI_MAX // P) * D], BF, tag="gbf", name=f"{name}_gbf")
                nc.scalar.activation(gbf[:, :nch * D], g_t[:, :nch * D], AF.Copy,
                                     scale=1.0 / SEL_SCALE)
                bc = 0
                for (w, ncw, bst, bsp) in batch:
                    bank_t, b, pA, pB = bank_halves(w)
                    for k in range(ncw):
                        c = bc + k
                        st = (bst and k == 0)
                        sp = (bsp and k == ncw - 1)
                        nc.tensor.matmul(out=pA, lhsT=sel_t[:, c * 2 * P: c * 2 * P + P],
                                         rhs=gbf[:, c * D:(c + 1) * D], start=st, stop=False,
                                         skip_group_check=True)
                        nc.tensor.matmul(out=pB, lhsT=sel_t[:, c * 2 * P + P: (c + 1) * 2 * P],
                                         rhs=gbf[:, c * D:(c + 1) * D], start=False, stop=sp,
                                         skip_group_check=True)
                    if bsp:
                        w0 = g * SGW + 4 * b
                        nwin = min(4, len(wps) - 4 * b) * 2
                        nc.vector.tensor_copy(
                            out_sb[:, (2 * w0) * D:(2 * w0) * D + nwin * D],
                            bank_t[:, :nwin * D])
                    bc += ncw
                chunk_pos += nch
        if sg_hook is not None:
            sg_hook(g)
    assert chunk_pos == layer["tch"], f"{name}: {chunk_pos} != {layer['tch']}"


def emit_epilogue(cx, name, f0c_dram, f1c_sb, f2_sb, acc_sb, w0, w1):
    """acc[w0:w1 windows] = f0c + f1c/max(||f1c||,eps) + f2/max(||f2||,eps)."""
    nc = cx.nc
    nw = w1 - w0
    sl = slice(w0 * D, w1 * D)
    f0c = cx.ep.tile([P, nw * D], F32, tag="ep_f0", name=f"{name}_f0")
    nc.sync.dma_start(out=f0c[:].rearrange("p (w d) -> p w d", w=nw),
                      in_=f0c_dram[w0 * P:w1 * P, :].rearrange("(w p) d -> p w d", p=P))

    def normed(src, tag):
        sq = cx.ep.tile([P, nw * D], F32, tag="ep_sq", name=f"{name}_{tag}_sq")
        nc.vector.tensor_mul(sq[:], src, src)
        ss = cx.ep.tile([P, nw], F32, tag="ep_ss", name=f"{name}_{tag}_ss")
        nc.vector.reduce_sum(ss[:], sq[:].rearrange("p (w d) -> p w d", w=nw),
                             axis=mybir.AxisListType.X)
        sn = cx.ep.tile([P, nw], F32, tag="ep_sn", name=f"{name}_{tag}_sn")
        nc.scalar.activation(sn[:], ss[:], AF.Sqrt)
        nc.vector.tensor_scalar_max(sn[:], sn[:], 1e-12)
        rn = cx.ep.tile([P, nw], F32, tag="ep_rn", name=f"{name}_{tag}_rn")
        nc.vector.reciprocal(rn[:], sn[:])
        ct = cx.ep.tile([P, nw * D], F32, tag=f"ep_ct{tag}", name=f"{name}_{tag}_ct")
        nc.vector.tensor_mul(ct[:].rearrange("p (w d) -> p w d", w=nw),
                             src.rearrange("p (w d) -> p w d", w=nw),
                             rn[:].to_broadcast([P, nw, D]))
        return ct

    c1 = normed(f1c_sb[:, sl], "n1")
    c2 = normed(f2_sb[:, sl], "n2")
    nc.vector.tensor_add(acc_sb[:, sl], f0c[:], c1[:])
    nc.vector.tensor_add(acc_sb[:, sl], acc_sb[:, sl], c2[:])


def indirect_gather_rows(cx, out_sb, table_ap, idx_sb, ncols):
    nc = cx.nc
    for k in range(ncols):
        nc.gpsimd.indirect_dma_start(
            out=out_sb[:, k * D:(k + 1) * D], out_offset=None,
            in_=table_ap,
            in_offset=bass.IndirectOffsetOnAxis(ap=idx_sb[:, k:k + 1], axis=0))


def normalize_rows(cx, x_sb, ngroups, tag):
    nc = cx.nc
    sq = cx.lp.tile([P, ngroups * D], F32, tag=f"{tag}_sq")
    nc.vector.tensor_mul(sq[:], x_sb[:, :ngroups * D], x_sb[:, :ngroups * D])
    ss = cx.lp.tile([P, ngroups], F32, tag=f"{tag}_ss")
    nc.vector.reduce_sum(ss[:], sq[:].rearrange("p (w d) -> p w d", w=ngroups),
                         axis=mybir.AxisListType.X)
    sn = cx.lp.tile([P, ngroups], F32, tag=f"{tag}_sn")
    nc.scalar.activation(sn[:], ss[:], AF.Sqrt)
    nc.vector.tensor_scalar_max(sn[:], sn[:], 1e-12)
    rn = cx.lp.tile([P, ngroups], F32, tag=f"{tag}_rn")
    nc.vector.reciprocal(rn[:], sn[:])
    nc.vector.tensor_mul(
        x_sb[:, :ngroups * D].rearrange("p (w d) -> p w d", w=ngroups),
        x_sb[:, :ngroups * D].rearrange("p (w d) -> p w d", w=ngroups),
        rn[:].to_broadcast([P, ngroups, D]))


def rowdot(cx, a_sb, b_sb, out_sb, ngroups, tag):
    nc = cx.nc
    t = cx.lp.tile([P, ngroups * D], F32, tag=f"{tag}_t")
    nc.vector.tensor_mul(t[:], a_sb[:, :ngroups * D], b_sb[:, :ngroups * D])
    nc.vector.reduce_sum(out_sb[:, :ngroups], t[:].rearrange("p (w d) -> p w d", w=ngroups),
                         axis=mybir.AxisListType.X)


def transpose_groups(cx, src_sb, ngroups, tag):
    nc = cx.nc
    out = cx.lp.tile([P, ngroups * P], F32, tag=f"{tag}_T")
    for g in range(ngroups):
        pt = cx.psp.tile([P, 512], F32, space="PSUM", tag="trp", bufs=1)
        nc.tensor.transpose(out=pt[:D, :P], in_=src_sb[:, g * D:(g + 1) * D],
                            identity=cx.ident[:])
        nc.vector.tensor_copy(out[:D, g * P:(g + 1) * P], pt[:D, :P])
    return out


def build(pp):
    c_il1, c_bl1 = pp["c_il1"], pp["c_bl1"]
    c_il2, c_bl2, c_ag = pp["c_il2"], pp["c_bl2"], pp["c_ag"]
    nw_il1, nw_bl1 = c_il1.R // P, c_bl1.R // P
    nw_il2, nw_bl2, nw_ag = c_il2.R // P, c_bl2.R // P, c_ag.R // P

    nc = bacc.Bacc("TRN2", target_bir_lowering=False, debug=False, num_devices=NCORES,
                   num_swdge_queues=4, dynamic_dma_scratch_size=32768)
    cx = Ctx()
    cx.nc = nc

    f0_il = nc.dram_tensor("f0_il", [NU + NI, D], F32, kind="ExternalInput")
    f0_bl = nc.dram_tensor("f0_bl", [NU + NB, D], F32, kind="ExternalInput")
    g_in = {}
    for nm in ("il1", "bl1", "il2", "bl2", "ag"):
        tch = pp[nm]["tch"]
        g_in[nm] = dict(
            idx=nc.dram_tensor(f"{nm}_idx", [P, tch * 8], I16, kind="ExternalInput"),
            sel=nc.dram_tensor(f"{nm}_sel", [P, tch * 2 * P], FP8, kind="ExternalInput"))
    f0c_il = nc.dram_tensor("f0c_il", [c_il2.R, D], F32, kind="ExternalInput")
    f0c_bl = nc.dram_tensor("f0c_bl", [c_bl2.R, D], F32, kind="ExternalInput")
    lidx = {k: nc.dram_tensor(f"loss_{k}", [P, v.shape[1]], I32, kind="ExternalInput")
            for k, v in pp["loss"][0].items()}
    aug_in = {k: nc.dram_tensor(k, [P, 16], I32, kind="ExternalInput")
              for k in ("aug_u_bl", "aug_b0_bl")}
    out_t = nc.dram_tensor("out", [1, 2], F32, kind="ExternalOutput")

    with tile.TileContext(nc) as tc:
        cx.tc = tc
        es = []
        def pool(name, bufs, **kw):
            p = tc.tile_pool(name=name, bufs=bufs, **kw)
            es.append(p)
            return p.__enter__()
        cx.psp = pool("psum", 1, space="PSUM")
        cx.dramp = pool("dram", 1, space="DRAM")
        cx.cp = pool("const", 1)
        cx.idxp = pool("gidx", 3)
        cx.selp = pool("gsel", 3)
        cx.gp = pool("gath", 2)
        cx.gbp = pool("gbf", 2)
        cx.ep = pool("epil", 1)
        cx.flp = pool("floc", 1)
        cx.lp = pool("loss", 1)

        iota_i = cx.cp.tile([P, P], I32)
        nc.gpsimd.iota(iota_i[:], pattern=[[1, P]], base=0, channel_multiplier=0)
        cx.ident = cx.cp.tile([P, P], F32)
        make_identity(nc, cx.ident[:])
        ones_col = cx.cp.tile([P, 1], F32)
        nc.vector.memset(ones_col[:], 1.0)

        flocal = cx.flp.tile([P, max(nw_il1, nw_bl1) * D], F32, tag="flocal",
                             name="flocal")
        acc_sb = cx.flp.tile([P, max(nw_il2, nw_bl2) * D], F32, tag="accsb",
                             name="accsb")
        f1c_il = cx.flp.tile([P, nw_il2 * D], F32, tag="f1c_il", name="f1c_il")
        f1c_bl = cx.flp.tile([P, nw_bl2 * D], F32, tag="f1c_bl", name="f1c_bl")

        def sharded_table(cm, nm):
            """DRAM tiles per shard + AG-in tile; returns (tiles, agin, windows).
            windows[s] = AP slice for source window s of the global gid space."""
            tiles = []
            for si, rows in enumerate(cm.shard_rows):
                tiles.append(cx.dramp.tile([NCORES * rows, D], F32,
                                           addr_space="Shared",
                                           tag=f"{nm}_t{si}", name=f"{nm}_t{si}"))
            agin = cx.dramp.tile([cm.R, D], F32, tag=f"{nm}_agin", name=f"{nm}_agin")
            windows = []
            off = 0
            for si, rows in enumerate(cm.shard_rows):
                gsz = NCORES * rows
                o = 0
                while o < gsz:
                    wr = min(SRC_WIN, gsz - o)
                    windows.append(tiles[si][o:o + wr, :])
                    o += wr
                off += gsz
            return tiles, agin, windows

        f1il_tiles, il1_agin, il2_windows = sharded_table(c_il1, "f1il")
        f1bl_tiles, bl1_agin, bl2_windows = sharded_table(c_bl1, "f1bl")
        ail_tiles, il2_agin, ag_windows = sharded_table(c_il2, "ail")
        abl_tiles, bl2_agin, _ = sharded_table(c_bl2, "abl")
        ilb_tiles, ag_agin, _ = sharded_table(c_ag, "ilb")

        def shard_ag(cm, flocal_sb, agin, tiles, si, extra_src=None):
            """DMA shard si's slots from flocal (or extra_src) and AllGather."""
            s0 = si * cm.shard_slots
            rows = cm.shard_rows[si]
            src = extra_src if extra_src is not None else flocal_sb
            nc.sync.dma_start(
                out=agin[s0:s0 + rows, :].rearrange("(w p) d -> p w d", p=P),
                in_=src[:, (s0 // P) * D:((s0 + rows) // P) * D]
                    .rearrange("p (w d) -> p w d", w=rows // P))
            nc.gpsimd.collective_compute(
                "AllGather", ALU.bypass, replica_groups=[list(range(NCORES))],
                ins=[agin[s0:s0 + rows, :].opt()], outs=[tiles[si][:].opt()])

        def last_sg_of_shard(cm, g, nsg):
            # shard si covers sgs [si*k, si*k+k) where k = shard_slots/sg_slots
            k = cm.shard_slots // (SGW * 2 * P)
            if g == nsg - 1:
                return len(cm.shard_rows) - 1
            if (g + 1) % k == 0:
                return g // k
            return None

        # ---- phase 1: L1 both graphs, sharded AGs inline
        nsg1 = len(pp["il1"]["prog"])

        def il1_hook(g):
            si = last_sg_of_shard(c_il1, g, nsg1)
            if si is not None:
                shard_ag(c_il1, flocal, il1_agin, f1il_tiles, si)

        emit_spmm(cx, "il1", pp["il1"],
                  [f0_il[s * SRC_WIN:min((s + 1) * SRC_WIN, NU + NI), :]
                   for s in range(pp["il1"]["nsrc"])],
                  g_in["il1"]["idx"], g_in["il1"]["sel"], flocal, il1_hook)
        nc.vector.tensor_copy(f1c_il[:], flocal[:, :nw_il2 * D])

        nsg2 = len(pp["bl1"]["prog"])

        def bl1_hook(g):
            si = last_sg_of_shard(c_bl1, g, nsg2)
            if si is not None:
                shard_ag(c_bl1, flocal, bl1_agin, f1bl_tiles, si)

        emit_spmm(cx, "bl1", pp["bl1"],
                  [f0_bl[s * SRC_WIN:min((s + 1) * SRC_WIN, NU + NB), :]
                   for s in range(pp["bl1"]["nsrc"])],
                  g_in["bl1"]["idx"], g_in["bl1"]["sel"], flocal, bl1_hook)
        nc.vector.tensor_copy(f1c_bl[:], flocal[:, :nw_bl2 * D])

        # ---- phase 2: L2 IL with per-shard epilogue + AG
        nsg3 = len(pp["il2"]["prog"])

        def il2_hook(g):
            si = last_sg_of_shard(c_il2, g, nsg3)
            if si is not None:
                w0 = si * c_il2.shard_slots // P
                w1 = w0 + c_il2.shard_rows[si] // P
                emit_epilogue(cx, f"il2s{si}", f0c_il, f1c_il, flocal, acc_sb,
                              w0, w1)
                shard_ag(c_il2, acc_sb, il2_agin, ail_tiles, si)

        emit_spmm(cx, "il2", pp["il2"], il2_windows, g_in["il2"]["idx"],
                  g_in["il2"]["sel"], flocal, il2_hook)

        # ---- phase 3: L2 BL
        nsg4 = len(pp["bl2"]["prog"])

        def bl2_hook(g):
            si = last_sg_of_shard(c_bl2, g, nsg4)
            if si is not None:
                w0 = si * c_bl2.shard_slots // P
                w1 = w0 + c_bl2.shard_rows[si] // P
                emit_epilogue(cx, f"bl2s{si}", f0c_bl, f1c_bl, flocal, acc_sb,
                              w0, w1)
                shard_ag(c_bl2, acc_sb, bl2_agin, abl_tiles, si)

        emit_spmm(cx, "bl2", pp["bl2"], bl2_windows, g_in["bl2"]["idx"],
                  g_in["bl2"]["sel"], flocal, bl2_hook)

        # ---- phase 4: agg spmm + AG
        nsg5 = len(pp["ag"]["prog"])

        def ag_hook(g):
            si = last_sg_of_shard(c_ag, g, nsg5)
            if si is not None:
                shard_ag(c_ag, flocal, ag_agin, ilb_tiles, si)

        emit_spmm(cx, "ag", pp["ag"], ag_windows, g_in["ag"]["idx"],
                  g_in["ag"]["sel"], flocal, ag_hook)

        # ---- phase 5: loss
        assert len(abl_tiles) == 1 and len(ilb_tiles) == 1
        bsh = BATCH // NCORES
        ng = bsh // P
        lidx_sb = {}
        for k, t in lidx.items():
            s = cx.lp.tile([P, t.shape[1]], I32, tag=f"li_{k}")
            nc.sync.dma_start(out=s[:], in_=t[:])
            lidx_sb[k] = s
        for k, t in aug_in.items():
            s = cx.lp.tile([P, 16], I32, tag=f"li_{k}")
            nc.sync.dma_start(out=s[:], in_=t[:])
            lidx_sb[k] = s

        ail_ap = ail_tiles[0][:]
        abl_ap = abl_tiles[0][:]
        ilb_ap = ilb_tiles[0][:]

        def gather(tag, ap, idxk, ncols):
            sb = cx.lp.tile([P, ncols * D], F32, tag=tag)
            indirect_gather_rows(cx, sb, ap, lidx_sb[idxk], ncols)
            return sb

        pos_u_il = gather("pos_u_il", ail_ap, "u_il", ng)
        pos_u_bl = gather("pos_u_bl", abl_ap, "u_bl", ng)
        b_bl0 = gather("b_bl0", abl_ap, "b_bl0", ng)
        b_bl1 = gather("b_bl1", abl_ap, "b_bl1", ng)
        aug_u = gather("aug_u", abl_ap, "aug_u_bl", 16)
        aug_b = gather("aug_b", abl_ap, "aug_b0_bl", 16)

        # normalized copy of pos_u_il for c1 (BPR later needs the raw rows)
        pos_u_il_n = cx.lp.tile([P, ng * D], F32, tag="pos_u_il_n")
        nc.vector.tensor_copy(pos_u_il_n[:], pos_u_il[:, :ng * D])
        normalize_rows(cx, pos_u_il_n, ng, "npu")
        normalize_rows(cx, aug_u, 16, "nau")
        normalize_rows(cx, aug_b, 16, "nab")

        part = cx.lp.tile([P, 4], F32, tag="parts")
        nc.vector.memset(part[:], 0.0)

        def closs_partial(pos_my, aug_full, aug_my_cols, out_col):
            posT = transpose_groups(cx, pos_my, ng, "pT")
            augT = transpose_groups(cx, aug_full, 16, "aT")
            ps = cx.lp.tile([P, ng], F32, tag="ps")
            rowdot(cx, pos_my, aug_my_cols, ps, ng, f"psd{out_col}")
            lse = cx.lp.tile([P, ng], F32, tag="lse")
            for g in range(ng):
                ttl_ps = cx.psp.tile([P, 512], F32, space="PSUM", tag="ttl", bufs=1)
                ttl = cx.lp.tile([P, BATCH], F32, tag="ttl")
                for nb_ in range(BATCH // 512):
                    nc.tensor.matmul(
                        out=ttl_ps[:, :512],
                        lhsT=posT[:D, g * P:(g + 1) * P],
                        rhs=augT[:D, nb_ * 512:(nb_ + 1) * 512],
                        start=True, stop=True)
                    nc.vector.tensor_copy(ttl[:, nb_ * 512:(nb_ + 1) * 512], ttl_ps[:, :512])
                mx = cx.lp.tile([P, 1], F32, tag="mx")
                nc.vector.reduce_max(mx[:], ttl[:].rearrange("p (w d) -> p w d", w=1),
                                     axis=mybir.AxisListType.X)
                nmx = cx.lp.tile([P, 1], F32, tag="nmx")
                nc.vector.tensor_scalar_mul(nmx[:], mx[:], -4.0)
                ex = cx.lp.tile([P, BATCH], F32, tag="ex")
                se = cx.lp.tile([P, 1], F32, tag="se")
                nc.scalar.activation(ex[:], ttl[:], AF.Exp, bias=nmx[:, :1], scale=4.0,
                                     accum_out=se[:, :1])
                ln = cx.lp.tile([P, 1], F32, tag="ln")
                nc.scalar.activation(ln[:], se[:], AF.Ln)
                m4 = cx.lp.tile([P, 1], F32, tag="m4")
                nc.vector.tensor_scalar_mul(m4[:], mx[:], 4.0)
                nc.vector.tensor_add(lse[:, g:g + 1], ln[:], m4[:])
            t4 = cx.lp.tile([P, ng], F32, tag="t4")
            nc.vector.tensor_scalar_mul(t4[:], ps[:], 4.0)
            nc.vector.tensor_tensor(out=t4[:], in0=t4[:], in1=lse[:], op=ALU.subtract)
            nc.vector.reduce_sum(part[:, out_col:out_col + 1],
                                 t4[:].rearrange("p (w d) -> p w d", w=1),
                                 axis=mybir.AxisListType.X)

        aug_u_my = gather("aug_u_my", abl_ap, "u_bl", ng)
        normalize_rows(cx, aug_u_my, ng, "naum")
        aug_b_my = gather("aug_b_my", abl_ap, "b_bl0", ng)
        normalize_rows(cx, aug_b_my, ng, "nabm")
        closs_partial(pos_u_il_n, aug_u, aug_u_my, 1)

        b_il0 = gather("b_il0", ilb_ap, "b_il0", ng)
        b_il1 = gather("b_il1", ilb_ap, "b_il1", ng)
        pr0 = cx.lp.tile([P, ng], F32, tag="pr0")
        pr1 = cx.lp.tile([P, ng], F32, tag="pr1")
        tmp = cx.lp.tile([P, ng], F32, tag="prt")
        rowdot(cx, pos_u_il, b_il0, pr0, ng, "d0")
        rowdot(cx, pos_u_bl, b_bl0, tmp, ng, "d1")
        nc.vector.tensor_add(pr0[:], pr0[:], tmp[:])
        rowdot(cx, pos_u_il, b_il1, pr1, ng, "d2")
        rowdot(cx, pos_u_bl, b_bl1, tmp, ng, "d3")
        nc.vector.tensor_add(pr1[:], pr1[:], tmp[:])
        x = cx.lp.tile([P, ng], F32, tag="bprx")
        nc.vector.tensor_tensor(out=x[:], in0=pr1[:], in1=pr0[:], op=ALU.subtract)
        negx = cx.lp.tile([P, ng], F32, tag="bprnx")
        nc.vector.tensor_scalar_mul(negx[:], x[:], -1.0)
        nax = cx.lp.tile([P, ng], F32, tag="bprax")
        nc.vector.tensor_tensor(out=nax[:], in0=x[:], in1=negx[:], op=ALU.min)
        e = cx.lp.tile([P, ng], F32, tag="bpre")
        nc.scalar.activation(e[:], nax[:], AF.Exp)
        nc.vector.tensor_scalar_add(e[:], e[:], 1.0)
        l1p = cx.lp.tile([P, ng], F32, tag="bprl")
        nc.scalar.activation(l1p[:], e[:], AF.Ln)
        sp = cx.lp.tile([P, ng], F32, tag="bprsp")
        nc.vector.tensor_scalar_max(sp[:], x[:], 0.0)
        nc.vector.tensor_add(sp[:], sp[:], l1p[:])
        nc.vector.reduce_sum(part[:, 0:1], sp[:].rearrange("p (w d) -> p w d", w=1),
                             axis=mybir.AxisListType.X)

        my_pos_b = cx.lp.tile([P, ng * D], F32, tag="my_pb")
        nc.vector.tensor_copy(my_pos_b[:], b_il0[:, :ng * D])
        normalize_rows(cx, my_pos_b, ng, "npb")
        closs_partial(my_pos_b, aug_b, aug_b_my, 2)

        pp_ps = cx.psp.tile([P, 512], F32, space="PSUM", tag="ppps", bufs=1)
        nc.tensor.matmul(out=pp_ps[:1, :4], lhsT=ones_col[:], rhs=part[:],
                         start=True, stop=True)
        psum_sb = cx.lp.tile([1, 4], F32, tag="psums")
        nc.vector.tensor_copy(psum_sb[:], pp_ps[:1, :4])
        ar_in = cx.dramp.tile([1, 4], F32, tag="ar_in")
        ar_out = cx.dramp.tile([1, 4], F32, addr_space="Shared", tag="ar_out")
        nc.sync.dma_start(out=ar_in[:], in_=psum_sb[:])
        nc.gpsimd.collective_compute(
            "AllReduce", ALU.add, replica_groups=[list(range(NCORES))],
            ins=[ar_in[:].opt()], outs=[ar_out[:].opt()])
        fin = cx.lp.tile([1, 4], F32, tag="fin")
        nc.sync.dma_start(out=fin[:], in_=ar_out[:])
        res = cx.lp.tile([1, 2], F32, tag="res")
        nc.vector.tensor_scalar_mul(res[:, 0:1], fin[:, 0:1], 1.0 / BATCH)
        t = cx.lp.tile([1, 1], F32, tag="rt")
        nc.vector.tensor_add(t[:], fin[:, 1:2], fin[:, 2:3])
        nc.vector.tensor_scalar_mul(res[:, 1:2], t[:], -0.5 / BATCH)
        nc.sync.dma_start(out=out_t[:], in_=res[:])

        for p in reversed(es):
            p.__exit__(None, None, None)
    nc.compile()
    return nc


# ---------------------------------------------------------------- entry point

def _install_ntff_hook():
    if "antenv.axon_hooks" in sys.modules:
        return
    try:
        mod = types.ModuleType("antenv.axon_hooks")
        _hook = [None]
        mod.set_axon_ntff_profile_hook = lambda h: _hook.__setitem__(0, h)
        mod.get_axon_ntff_profile_hook = lambda: _hook[0]
        sys.modules["antenv.axon_hooks"] = mod
        import antenv
        antenv.axon_hooks = mod
        from trn_agent_boot.trn_boot import _ntff_profile_via_ctypes
        hook = _ntff_profile_via_ctypes("/opt/axon/libaxon_pjrt.so")
        if hook is not None:
            mod.set_axon_ntff_profile_hook(hook)
    except Exception:
        pass


def make_in_maps(pp):
    maps = []
    for c in range(NCORES):
        m = {
            "f0_il": pp["f0_il"], "f0_bl": pp["f0_bl"],
            "f0c_il": pp["f0c_il"][c], "f0c_bl": pp["f0c_bl"][c],
            "aug_u_bl": pp["aug_u_bl"], "aug_b0_bl": pp["aug_b0_bl"],
        }
        for nm in ("il1", "bl1", "il2", "bl2", "ag"):
            m[f"{nm}_idx"] = pp[nm]["idx"][c]
            m[f"{nm}_sel"] = pp[nm]["sel"][c]
        for k, v in pp["loss"][c].items():
            m[f"loss_{k}"] = v
        maps.append(m)
    return maps


_CACHE = {}


def kernel(**inputs) -> np.ndarray:
    _install_ntff_hook()
    pp = preprocess(inputs)
    key = "full"
    if key not in _CACHE:
        _CACHE[key] = build(pp)
    nc = _CACHE[key]
    in_maps = make_in_maps(pp)
    trace = bool(int(os.environ.get("DSCBR_TRACE", "0")))
    res = run_bass_kernel_spmd(nc, in_maps, core_ids=list(range(NCORES)), trace=trace)
    if trace and res.exec_time_ns:
        print(f"HW exec time: {res.exec_time_ns} ns")
    out = res.results[0]["out"].reshape(2).astype(np.float32)
    return out


# revision 10
# speedup vs baseline: 1.4957x; 1.2019x over previous
"""Trainium2 Bass kernel for nn_DSCBR (gnn_message_passing), v3.

Strategy (8 NeuronCores, SPMD):
- Recursive 2-hop trimming: only rows observable in the final losses are
  computed (6.8M -> 3.6M edges).
- SpMM: dma_gather of source rows (int16-windowed) + selection-matrix
  matmuls. Selection matrices are HOST-BUILT fp8 (e4m3) with edge values
  and layer scale baked in (x32 to dodge subnormals; undone in the
  f32->bf16 cast on the Scalar engine).
- PSUM: one accumulation group per 2KB bank; 4 win-pairs share a bank
  (start zeroes the bank lazily); win-pairs stay resident across all
  source windows -> one [128,512] drain per bank per layer.
- Sharded AllGathers: compact ids are shard-major so f1/acc tables are
  all-gathered in slices as soon as their supergroups drain, overlapping
  the remaining gathers.
- L2 dest rows are a prefix of the L1 dest rows per core, so the
  epilogue's f1 rows are just a copy of the local L1 output in SBUF.
- Loss computed batch-sharded on compact tables + tiny AllReduce.
"""
import os
import sys
import types

sys.path.insert(0, "/opt/trn_rl_repo")

import numpy as np

import concourse.bass as bass
import concourse.bacc as bacc
import concourse.mybir as mybir
import concourse.tile as tile
from concourse.bass_utils import run_bass_kernel_spmd
from concourse.masks import make_identity

P = 128
NCORES = 8
SRC_WIN = 32768
GI_MAX = 1920          # idxs/call: 121 descs fits 8KB rings (4-queue split)
SGW = 16               # win-pairs per PSUM supergroup (4 banks)
D = 64
NU, NI, NB = 100000, 50000, 20000
BATCH = 2048
SEL_SCALE = 32.0
F32 = mybir.dt.float32
BF = mybir.dt.bfloat16
FP8 = mybir.dt.float8e4
I32 = mybir.dt.int32
I16 = mybir.dt.int16
AF = mybir.ActivationFunctionType
ALU = mybir.AluOpType


def cdiv(a, b):
    return -(-a // b)


# ---------------------------------------------------------------- host prep

def wrap_idx16(flat):
    return np.ascontiguousarray(np.tile(flat.reshape(-1, 16).T.astype(np.int16), (8, 1)))


def idx_cols_i32(flat):
    n = flat.shape[0]
    assert n % P == 0
    return np.ascontiguousarray(flat.reshape(-1, P).T.astype(np.int32))


class Compact:
    """Shard-major compact numbering of a row set dealt across cores.

    Slots per core are grouped into shards of `shard_slots`; global id =
    shard_off[s] + core*shard_rows[s] + (slot - s*shard_slots).
    """

    def __init__(self, pref_rows_percore, extra_rows, n_space, shard_slots):
        rows_pc = [list(x) for x in pref_rows_percore]
        for j, r in enumerate(extra_rows):
            rows_pc[j % NCORES].append(int(r))
        per = max(len(x) for x in rows_pc)
        self.R = cdiv(per, 2 * P) * (2 * P)
        self.shard_slots = shard_slots
        ns = cdiv(self.R, shard_slots)
        self.shard_rows = [min(shard_slots, self.R - s * shard_slots)
                           for s in range(ns)]
        self.shard_off = np.concatenate(
            [[0], np.cumsum([NCORES * r for r in self.shard_rows])]).astype(np.int64)
        self.V = int(self.shard_off[-1])
        self.core_rows = [np.asarray(x, np.int64) for x in rows_pc]
        self.comp_core = np.full(n_space, -1, np.int64)
        self.comp_slot = np.full(n_space, -1, np.int64)
        for c in range(NCORES):
            rr = self.core_rows[c]
            self.comp_core[rr] = c
            self.comp_slot[rr] = np.arange(len(rr))

    def gid_of_slot(self, core, slot):
        s = slot // self.shard_slots
        return (self.shard_off[s] + core * np.asarray(self.shard_rows)[s]
                + (slot - s * self.shard_slots))

    def gid(self, rows):
        core = self.comp_core[rows]
        slot = self.comp_slot[rows]
        assert (core >= 0).all()
        return self.gid_of_slot(core, slot)


def build_layer(dest_core, dest_slot, src_idx, val, R_out, V_src, scale):
    """Host-built per-core idx + fp8 sel streams and the emission program."""
    E = len(dest_core)
    core = dest_core
    wp = dest_slot // (2 * P)
    lrow = dest_slot % (2 * P)
    swin = src_idx // SRC_WIN
    sidx = src_idx % SRC_WIN
    nwp = R_out // (2 * P)
    nsrc = cdiv(V_src, SRC_WIN)
    nsg = cdiv(nwp, SGW)

    # Dedup: edges in the same (core, swin, wp) group with the same source
    # share one gather slot; their sel entries merge (sum on collision).
    order = np.lexsort((sidx, wp, swin, core))
    c_s, s_s, w_s, l_s, si_s, v_s = (core[order], swin[order], wp[order],
                                     lrow[order], sidx[order], val[order])
    gkey = (c_s * nsrc + s_s) * nwp + w_s
    new_g = np.concatenate([[True], gkey[1:] != gkey[:-1]])
    new_u = new_g | np.concatenate([[True], si_s[1:] != si_s[:-1]])
    uidx = np.cumsum(new_u) - 1
    grp_first_uidx = np.maximum.accumulate(np.where(new_g, uidx, 0))
    slot_in_grp = uidx - grp_first_uidx

    counts = np.zeros((NCORES, nsrc, nwp), np.int64)
    np.add.at(counts, (c_s[new_u], s_s[new_u], w_s[new_u]), 1)
    nchunks = cdiv(np.max(counts, axis=0), P)          # [nsrc, nwp]
    for w in range(nwp):
        if nchunks[:, w].sum() == 0:
            nchunks[0, w] = 1

    chunk_off = np.zeros((nsrc, nwp), np.int64)
    pos = 0
    for g in range(nsg):
        wps = range(g * SGW, min((g + 1) * SGW, nwp))
        for s in range(nsrc):
            for w in wps:
                chunk_off[s, w] = pos
                pos += nchunks[s, w]
    tch = pos

    epos = chunk_off[s_s, w_s] * P + slot_in_grp

    import ml_dtypes
    idx_streams = np.zeros((NCORES, tch * P), np.int16)
    sel8 = []
    for c in range(NCORES):
        m = c_s == c
        idx_streams[c, epos[m]] = si_s[m]
        selc = np.zeros((tch * P, 2 * P), np.float32)
        np.add.at(selc, (epos[m], l_s[m]), v_s[m] * scale)
        selc = selc.astype(ml_dtypes.float8_e4m3fn)
        sel8.append(np.ascontiguousarray(
            selc.reshape(tch, P, 2 * P).transpose(1, 0, 2).reshape(P, tch * 2 * P)))
    idx8 = [wrap_idx16(idx_streams[c]) for c in range(NCORES)]

    # Program: one PSUM accumulation group per 2KB bank (4 consecutive
    # win-pairs); start on the bank's first matmul of the sg, stop+drain on
    # its last.
    prog = []
    for g in range(nsg):
        wps = list(range(g * SGW, min((g + 1) * SGW, nwp)))
        flat = []
        for s in range(nsrc):
            for w in wps:
                take = int(nchunks[s, w])
                if take:
                    flat.append([s, w, take, False, False])
        bank_of = lambda w: (w - g * SGW) // 4
        for b in set(bank_of(it[1]) for it in flat):
            idxs = [i for i, it in enumerate(flat) if bank_of(it[1]) == b]
            flat[idxs[0]][3] = True
            flat[idxs[-1]][4] = True
        sws = []
        fi_pos = 0
        for s in range(nsrc):
            batches = []
            cur, cur_n = [], 0
            while fi_pos < len(flat) and flat[fi_pos][0] == s:
                _, w, take, st, sp = flat[fi_pos]
                fi_pos += 1
                while take > 0:
                    room = GI_MAX // P - cur_n
                    if room == 0:
                        batches.append(cur)
                        cur, cur_n = [], 0
                        room = GI_MAX // P
                    t = min(room, take)
                    cur.append((w, t, st, sp and t == take))
                    st = False
                    cur_n += t
                    take -= t
            if cur:
                batches.append(cur)
            if batches:
                sws.append((s, batches))
        prog.append((g, wps, sws))
    return dict(idx=idx8, sel=sel8, prog=prog, tch=tch, nwp=nwp, nsrc=nsrc,
                V_src=V_src)


def preprocess(inputs):
    u = np.asarray(inputs["users_feature"], np.float32)
    it = np.asarray(inputs["items_feature"], np.float32)
    b = np.asarray(inputs["bundles_feature"], np.float32)
    il_row = np.asarray(inputs["il_row"]).astype(np.int64)
    il_col = np.asarray(inputs["il_col"]).astype(np.int64)
    il_val = np.asarray(inputs["il_val"], np.float32)
    bl_row = np.asarray(inputs["bl_row"]).astype(np.int64)
    bl_col = np.asarray(inputs["bl_col"]).astype(np.int64)
    bl_val = np.asarray(inputs["bl_val"], np.float32)
    agg_row = np.asarray(inputs["agg_row"]).astype(np.int64)
    agg_col = np.asarray(inputs["agg_col"]).astype(np.int64)
    agg_val = np.asarray(inputs["agg_val"], np.float32)
    users = np.asarray(inputs["users"]).astype(np.int64)
    bundles = np.asarray(inputs["bundles"]).astype(np.int64)

    N1, N2 = NU + NI, NU + NB

    # ---- need sets (2-hop trimming)
    uu = np.unique(users)
    need_b = np.unique(bundles)
    bmask = np.zeros(NB, bool); bmask[need_b] = True
    agg_keep = bmask[agg_row]
    need_i = np.unique(agg_col[agg_keep])
    umask = np.zeros(NU, bool); umask[uu] = True
    imask = np.zeros(NI, bool); imask[need_i] = True

    m_il2 = np.concatenate([umask, imask])
    e_il2 = m_il2[il_row]
    need_f1_il = np.zeros(N1, bool)
    need_f1_il[np.unique(il_col[e_il2])] = True
    need_f1_il |= m_il2
    e_il1 = need_f1_il[il_row]

    m_bl2 = np.concatenate([umask, bmask])
    e_bl2 = m_bl2[bl_row]
    need_f1_bl = np.zeros(N2, bool)
    need_f1_bl[np.unique(bl_col[e_bl2])] = True
    need_f1_bl |= m_bl2
    e_bl1 = need_f1_bl[bl_row]

    # ---- compact maps: L2 sets first; their per-core rows become the slot
    # prefix of the L1 compaction (so f1 rows for the epilogue are local).
    SH1 = 2 * SGW * 2 * P      # 8192 slots/core per shard (2 sgs)
    SH2 = SGW * 2 * P          # 4096 (1 sg)
    # users first in the IL acc table so loss rows live in shard 0
    u_pref = [uu[c::NCORES] for c in range(NCORES)]
    m_il2_rest = m_il2.copy()
    m_il2_rest[uu] = False
    c_il2 = Compact(u_pref, np.nonzero(m_il2_rest)[0], N1, SH2)
    c_bl2 = Compact([[]] * NCORES, np.nonzero(m_bl2)[0], N2, SH2)
    rest_il1 = np.nonzero(need_f1_il & ~m_il2)[0]
    rest_bl1 = np.nonzero(need_f1_bl & ~m_bl2)[0]
    c_il1 = Compact(c_il2.core_rows, rest_il1, N1, SH1)
    c_bl1 = Compact(c_bl2.core_rows, rest_bl1, N2, SH1)
    c_ag = Compact([[]] * NCORES, need_b, NB, SH2)

    # ---- layer streams
    def layer(c_dst, erows, ecols_gid, evals, V_src, scale):
        return build_layer(c_dst.comp_core[erows], c_dst.comp_slot[erows],
                           ecols_gid, evals, c_dst.R, V_src, scale)

    il1 = layer(c_il1, il_row[e_il1], il_col[e_il1], il_val[e_il1], N1, 0.5)
    bl1 = layer(c_bl1, bl_row[e_bl1], bl_col[e_bl1], bl_val[e_bl1], N2, 0.5)
    il2 = layer(c_il2, il_row[e_il2], c_il1.gid(il_col[e_il2]), il_val[e_il2],
                c_il1.V, 1.0 / 3.0)
    bl2 = layer(c_bl2, bl_row[e_bl2], c_bl1.gid(bl_col[e_bl2]), bl_val[e_bl2],
                c_bl1.V, 1.0 / 3.0)
    agr, agc, agv = agg_row[agg_keep], agg_col[agg_keep], agg_val[agg_keep]
    ag = layer(c_ag, agr, c_il2.gid(NU + agc), agv, c_il2.V, 1.0)

    # ---- f0 tables and per-core compact-f0 epilogue inputs
    f0_il = np.concatenate([u, it], 0)
    f0_bl = np.concatenate([u, b], 0)

    def f0c(comp, R, f0):
        out = []
        for c in range(NCORES):
            rows = comp.core_rows[c]
            fc = np.zeros((R, D), np.float32)
            fc[:len(rows)] = f0[rows]
            out.append(fc)
        return out

    f0c_il = f0c(c_il2, c_il2.R, f0_il)
    f0c_bl = f0c(c_bl2, c_bl2.R, f0_bl)

    # ---- loss index maps
    loss = {}
    bsh = BATCH // NCORES
    for c in range(NCORES):
        sl = slice(c * bsh, (c + 1) * bsh)
        loss[c] = dict(
            u_il=idx_cols_i32(c_il2.gid(users[sl])),
            u_bl=idx_cols_i32(c_bl2.gid(users[sl])),
            b_il0=idx_cols_i32(c_ag.gid(bundles[sl, 0])),
            b_il1=idx_cols_i32(c_ag.gid(bundles[sl, 1])),
            b_bl0=idx_cols_i32(c_bl2.gid(bundles[sl, 0] + NU)),
            b_bl1=idx_cols_i32(c_bl2.gid(bundles[sl, 1] + NU)),
        )
    aug_u_bl = idx_cols_i32(c_bl2.gid(users))
    aug_b0_bl = idx_cols_i32(c_bl2.gid(bundles[:, 0] + NU))

    return dict(f0_il=f0_il, f0_bl=f0_bl,
                il1=il1, bl1=bl1, il2=il2, bl2=bl2, ag=ag,
                f0c_il=f0c_il, f0c_bl=f0c_bl,
                loss=loss, aug_u_bl=aug_u_bl, aug_b0_bl=aug_b0_bl,
                c_il1=c_il1, c_bl1=c_bl1, c_il2=c_il2, c_bl2=c_bl2, c_ag=c_ag)


# ---------------------------------------------------------------- bass build

class Ctx:
    pass


def emit_spmm(cx, name, layer, windows, idx_dram, sel_dram, out_sb, sg_hook=None):
    """One SpMM layer. windows: swin -> AP of the source rows.
    out_sb[:, w*64:(w+1)*64] receives dest window w. sg_hook(g) is called
    after supergroup g's drains are emitted. Gather calls alternate between
    two SWDGE queues so one ring drains while the other fills."""
    nc = cx.nc
    chunk_pos = 0
    for (g, wps, sws) in layer["prog"]:
        banks = {}

        def bank_halves(w):
            b = (w - g * SGW) // 4
            if b not in banks:
                banks[b] = cx.psp.tile([P, 512], F32, space="PSUM",
                                       tag=f"bank{b}", name=f"bank{b}", bufs=1)
            t = banks[b]
            off = ((w - g * SGW) % 4) * 2 * D
            return t, b, t[:, off:off + D], t[:, off + D:off + 2 * D]

        for (s, batches) in sws:
            src_slice = windows[s]
            for batch in batches:
                nch = sum(it[1] for it in batch)
                gi = nch * P
                idx_t = cx.idxp.tile([P, GI_MAX // 16], I16, tag="gidx", name=f"{name}_gidx")
                nc.sync.dma_start(out=idx_t[:, :gi // 16],
                                  in_=idx_dram[:, chunk_pos * 8: chunk_pos * 8 + gi // 16])
                sel_t = cx.selp.tile([P, (GI_MAX // P) * 2 * P], FP8, tag="gsel",
                                     name=f"{name}_gsel")
                nc.sync.dma_start(
                    out=sel_t[:, :nch * 2 * P],
                    in_=sel_dram[:, chunk_pos * 2 * P: (chunk_pos + nch) * 2 * P])
                g_t = cx.gp.tile([P, (GI_MAX // P) * D], F32, tag="gg", name=f"{name}_gg")
                cx.gq = (getattr(cx, "gq", 3) + 1) % 4
                nc.gpsimd.dma_gather(
                    out_ap=g_t[:, :nch * D].rearrange("p (c d) -> p c d", c=nch),
                    in_ap=src_slice,
                    idxs_ap=idx_t[:, :gi // 16],
                    num_idxs=gi, num_idxs_reg=gi, elem_size=D,
                    single_packet=False, queue_num=cx.gq)
                gbf = cx.gbp.tile([P, (GI_MAX // P) * D], BF, tag="gbf", name=f"{name}_gbf")
                nc.scalar.activation(gbf[:, :nch * D], g_t[:, :nch * D], AF.Copy,
                                     scale=1.0 / SEL_SCALE)
                bc = 0
                for (w, ncw, bst, bsp) in batch:
                    bank_t, b, pA, pB = bank_halves(w)
                    for k in range(ncw):
                        c = bc + k
                        st = (bst and k == 0)
                        sp = (bsp and k == ncw - 1)
                        nc.tensor.matmul(out=pA, lhsT=sel_t[:, c * 2 * P: c * 2 * P + P],
                                         rhs=gbf[:, c * D:(c + 1) * D], start=st, stop=False,
                                         skip_group_check=True)
                        nc.tensor.matmul(out=pB, lhsT=sel_t[:, c * 2 * P + P: (c + 1) * 2 * P],
                                         rhs=gbf[:, c * D:(c + 1) * D], start=False, stop=sp,
                                         skip_group_check=True)
                    if bsp:
                        w0 = g * SGW + 4 * b
                        nwin = min(4, len(wps) - 4 * b) * 2
                        nc.vector.tensor_copy(
                            out_sb[:, (2 * w0) * D:(2 * w0) * D + nwin * D],
                            bank_t[:, :nwin * D])
                    bc += ncw
                chunk_pos += nch
        if sg_hook is not None:
            sg_hook(g)
    assert chunk_pos == layer["tch"], f"{name}: {chunk_pos} != {layer['tch']}"


def emit_epilogue(cx, name, f0c_dram, f1c_sb, f2_sb, acc_sb, w0, w1):
    """acc[w0:w1 windows] = f0c + f1c/max(||f1c||,eps) + f2/max(||f2||,eps)."""
    nc = cx.nc
    nw = w1 - w0
    sl = slice(w0 * D, w1 * D)
    f0c = cx.ep.tile([P, nw * D], F32, tag="ep_f0", name=f"{name}_f0")
    nc.sync.dma_start(out=f0c[:].rearrange("p (w d) -> p w d", w=nw),
                      in_=f0c_dram[w0 * P:w1 * P, :].rearrange("(w p) d -> p w d", p=P))

    def normed(src, tag):
        sq = cx.ep.tile([P, nw * D], F32, tag="ep_sq", name=f"{name}_{tag}_sq")
        nc.vector.tensor_mul(sq[:], src, src)
        ss = cx.ep.tile([P, nw], F32, tag="ep_ss", name=f"{name}_{tag}_ss")
        nc.vector.reduce_sum(ss[:], sq[:].rearrange("p (w d) -> p w d", w=nw),
                             axis=mybir.AxisListType.X)
        sn = cx.ep.tile([P, nw], F32, tag="ep_sn", name=f"{name}_{tag}_sn")
        nc.scalar.activation(sn[:], ss[:], AF.Sqrt)
        nc.vector.tensor_scalar_max(sn[:], sn[:], 1e-12)
        rn = cx.ep.tile([P, nw], F32, tag="ep_rn", name=f"{name}_{tag}_rn")
        nc.vector.reciprocal(rn[:], sn[:])
        ct = cx.ep.tile([P, nw * D], F32, tag=f"ep_ct{tag}", name=f"{name}_{tag}_ct")
        nc.vector.tensor_mul(ct[:].rearrange("p (w d) -> p w d", w=nw),
                             src.rearrange("p (w d) -> p w d", w=nw),
                             rn[:].to_broadcast([P, nw, D]))
        return ct

    c1 = normed(f1c_sb[:, sl], "n1")
    c2 = normed(f2_sb[:, sl], "n2")
    nc.vector.tensor_add(acc_sb[:, sl], f0c[:], c1[:])
    nc.vector.tensor_add(acc_sb[:, sl], acc_sb[:, sl], c2[:])


def indirect_gather_rows(cx, out_sb, table_ap, idx_sb, ncols):
    nc = cx.nc
    for k in range(ncols):
        nc.gpsimd.indirect_dma_start(
            out=out_sb[:, k * D:(k + 1) * D], out_offset=None,
            in_=table_ap,
            in_offset=bass.IndirectOffsetOnAxis(ap=idx_sb[:, k:k + 1], axis=0))


def normalize_rows(cx, x_sb, ngroups, tag):
    nc = cx.nc
    sq = cx.lp.tile([P, ngroups * D], F32, tag=f"{tag}_sq")
    nc.vector.tensor_mul(sq[:], x_sb[:, :ngroups * D], x_sb[:, :ngroups * D])
    ss = cx.lp.tile([P, ngroups], F32, tag=f"{tag}_ss")
    nc.vector.reduce_sum(ss[:], sq[:].rearrange("p (w d) -> p w d", w=ngroups),
                         axis=mybir.AxisListType.X)
    sn = cx.lp.tile([P, ngroups], F32, tag=f"{tag}_sn")
    nc.scalar.activation(sn[:], ss[:], AF.Sqrt)
    nc.vector.tensor_scalar_max(sn[:], sn[:], 1e-12)
    rn = cx.lp.tile([P, ngroups], F32, tag=f"{tag}_rn")
    nc.vector.reciprocal(rn[:], sn[:])
    nc.vector.tensor_mul(
        x_sb[:, :ngroups * D].rearrange("p (w d) -> p w d", w=ngroups),
        x_sb[:, :ngroups * D].rearrange("p (w d) -> p w d", w=ngroups),
        rn[:].to_broadcast([P, ngroups, D]))


def rowdot(cx, a_sb, b_sb, out_sb, ngroups, tag):
    nc = cx.nc
    t = cx.lp.tile([P, ngroups * D], F32, tag=f"{tag}_t")
    nc.vector.tensor_mul(t[:], a_sb[:, :ngroups * D], b_sb[:, :ngroups * D])
    nc.vector.reduce_sum(out_sb[:, :ngroups], t[:].rearrange("p (w d) -> p w d", w=ngroups),
                         axis=mybir.AxisListType.X)


def transpose_groups(cx, src_sb, ngroups, tag):
    nc = cx.nc
    out = cx.lp.tile([P, ngroups * P], F32, tag=f"{tag}_T")
    for g in range(ngroups):
        pt = cx.psp.tile([P, 512], F32, space="PSUM", tag="trp", bufs=1)
        nc.tensor.transpose(out=pt[:D, :P], in_=src_sb[:, g * D:(g + 1) * D],
                            identity=cx.ident[:])
        nc.vector.tensor_copy(out[:D, g * P:(g + 1) * P], pt[:D, :P])
    return out


def build(pp):
    c_il1, c_bl1 = pp["c_il1"], pp["c_bl1"]
    c_il2, c_bl2, c_ag = pp["c_il2"], pp["c_bl2"], pp["c_ag"]
    nw_il1, nw_bl1 = c_il1.R // P, c_bl1.R // P
    nw_il2, nw_bl2, nw_ag = c_il2.R // P, c_bl2.R // P, c_ag.R // P

    nc = bacc.Bacc("TRN2", target_bir_lowering=False, debug=False, num_devices=NCORES,
                   num_swdge_queues=4, dynamic_dma_scratch_size=32768)
    cx = Ctx()
    cx.nc = nc

    f0_il = nc.dram_tensor("f0_il", [NU + NI, D], F32, kind="ExternalInput")
    f0_bl = nc.dram_tensor("f0_bl", [NU + NB, D], F32, kind="ExternalInput")
    g_in = {}
    for nm in ("il1", "bl1", "il2", "bl2", "ag"):
        tch = pp[nm]["tch"]
        g_in[nm] = dict(
            idx=nc.dram_tensor(f"{nm}_idx", [P, tch * 8], I16, kind="ExternalInput"),
            sel=nc.dram_tensor(f"{nm}_sel", [P, tch * 2 * P], FP8, kind="ExternalInput"))
    f0c_il = nc.dram_tensor("f0c_il", [c_il2.R, D], F32, kind="ExternalInput")
    f0c_bl = nc.dram_tensor("f0c_bl", [c_bl2.R, D], F32, kind="ExternalInput")
    lidx = {k: nc.dram_tensor(f"loss_{k}", [P, v.shape[1]], I32, kind="ExternalInput")
            for k, v in pp["loss"][0].items()}
    aug_in = {k: nc.dram_tensor(k, [P, 16], I32, kind="ExternalInput")
              for k in ("aug_u_bl", "aug_b0_bl")}
    out_t = nc.dram_tensor("out", [1, 2], F32, kind="ExternalOutput")

    with tile.TileContext(nc) as tc:
        cx.tc = tc
        es = []
        def pool(name, bufs, **kw):
            p = tc.tile_pool(name=name, bufs=bufs, **kw)
            es.append(p)
            return p.__enter__()
        cx.psp = pool("psum", 1, space="PSUM")
        cx.dramp = pool("dram", 1, space="DRAM")
        cx.cp = pool("const", 1)
        cx.idxp = pool("gidx", 4)
        cx.selp = pool("gsel", 4)
        cx.gp = pool("gath", 3)
        cx.gbp = pool("gbf", 3)
        cx.ep = pool("epil", 1)
        cx.flp = pool("floc", 1)
        cx.lp = pool("loss", 1)

        iota_i = cx.cp.tile([P, P], I32)
        nc.gpsimd.iota(iota_i[:], pattern=[[1, P]], base=0, channel_multiplier=0)
        cx.ident = cx.cp.tile([P, P], F32)
        make_identity(nc, cx.ident[:])
        ones_col = cx.cp.tile([P, 1], F32)
        nc.vector.memset(ones_col[:], 1.0)

        flocal = cx.flp.tile([P, max(nw_il1, nw_bl1) * D], F32, tag="flocal",
                             name="flocal")
        acc_sb = cx.flp.tile([P, max(nw_il2, nw_bl2) * D], F32, tag="accsb",
                             name="accsb")
        f1c_il = cx.flp.tile([P, nw_il2 * D], F32, tag="f1c_il", name="f1c_il")
        f1c_bl = cx.flp.tile([P, nw_bl2 * D], F32, tag="f1c_bl", name="f1c_bl")

        def sharded_table(cm, nm):
            """DRAM tiles per shard + AG-in tile; returns (tiles, agin, windows).
            windows[s] = AP slice for source window s of the global gid space."""
            tiles = []
            for si, rows in enumerate(cm.shard_rows):
                tiles.append(cx.dramp.tile([NCORES * rows, D], F32,
                                           addr_space="Shared",
                                           tag=f"{nm}_t{si}", name=f"{nm}_t{si}"))
            agin = cx.dramp.tile([cm.R, D], F32, tag=f"{nm}_agin", name=f"{nm}_agin")
            windows = []
            off = 0
            for si, rows in enumerate(cm.shard_rows):
                gsz = NCORES * rows
                o = 0
                while o < gsz:
                    wr = min(SRC_WIN, gsz - o)
                    windows.append(tiles[si][o:o + wr, :])
                    o += wr
                off += gsz
            return tiles, agin, windows

        f1il_tiles, il1_agin, il2_windows = sharded_table(c_il1, "f1il")
        f1bl_tiles, bl1_agin, bl2_windows = sharded_table(c_bl1, "f1bl")
        ail_tiles, il2_agin, ag_windows = sharded_table(c_il2, "ail")
        abl_tiles, bl2_agin, _ = sharded_table(c_bl2, "abl")
        ilb_tiles, ag_agin, _ = sharded_table(c_ag, "ilb")

        def shard_ag(cm, flocal_sb, agin, tiles, si, extra_src=None):
            """DMA shard si's slots from flocal (or extra_src) and AllGather."""
            s0 = si * cm.shard_slots
            rows = cm.shard_rows[si]
            src = extra_src if extra_src is not None else flocal_sb
            nc.sync.dma_start(
                out=agin[s0:s0 + rows, :].rearrange("(w p) d -> p w d", p=P),
                in_=src[:, (s0 // P) * D:((s0 + rows) // P) * D]
                    .rearrange("p (w d) -> p w d", w=rows // P))
            nc.gpsimd.collective_compute(
                "AllGather", ALU.bypass, replica_groups=[list(range(NCORES))],
                ins=[agin[s0:s0 + rows, :].opt()], outs=[tiles[si][:].opt()])

        def last_sg_of_shard(cm, g, nsg):
            # shard si covers sgs [si*k, si*k+k) where k = shard_slots/sg_slots
            k = cm.shard_slots // (SGW * 2 * P)
            if g == nsg - 1:
                return len(cm.shard_rows) - 1
            if (g + 1) % k == 0:
                return g // k
            return None

        # ---- phase 1: L1 both graphs, sharded AGs inline
        nsg1 = len(pp["il1"]["prog"])

        def il1_hook(g):
            si = last_sg_of_shard(c_il1, g, nsg1)
            if si is not None:
                shard_ag(c_il1, flocal, il1_agin, f1il_tiles, si)

        emit_spmm(cx, "il1", pp["il1"],
                  [f0_il[s * SRC_WIN:min((s + 1) * SRC_WIN, NU + NI), :]
                   for s in range(pp["il1"]["nsrc"])],
                  g_in["il1"]["idx"], g_in["il1"]["sel"], flocal, il1_hook)
        nc.vector.tensor_copy(f1c_il[:], flocal[:, :nw_il2 * D])

        nsg2 = len(pp["bl1"]["prog"])

        def bl1_hook(g):
            si = last_sg_of_shard(c_bl1, g, nsg2)
            if si is not None:
                shard_ag(c_bl1, flocal, bl1_agin, f1bl_tiles, si)

        emit_spmm(cx, "bl1", pp["bl1"],
                  [f0_bl[s * SRC_WIN:min((s + 1) * SRC_WIN, NU + NB), :]
                   for s in range(pp["bl1"]["nsrc"])],
                  g_in["bl1"]["idx"], g_in["bl1"]["sel"], flocal, bl1_hook)
        nc.vector.tensor_copy(f1c_bl[:], flocal[:, :nw_bl2 * D])

        # ---- phase 2: L2 IL with per-shard epilogue + AG
        nsg3 = len(pp["il2"]["prog"])

        def il2_hook(g):
            si = last_sg_of_shard(c_il2, g, nsg3)
            if si is not None:
                w0 = si * c_il2.shard_slots // P
                w1 = w0 + c_il2.shard_rows[si] // P
                emit_epilogue(cx, f"il2s{si}", f0c_il, f1c_il, flocal, acc_sb,
                              w0, w1)
                shard_ag(c_il2, acc_sb, il2_agin, ail_tiles, si)

        emit_spmm(cx, "il2", pp["il2"], il2_windows, g_in["il2"]["idx"],
                  g_in["il2"]["sel"], flocal, il2_hook)

        # ---- phase 3: L2 BL
        nsg4 = len(pp["bl2"]["prog"])

        def bl2_hook(g):
            si = last_sg_of_shard(c_bl2, g, nsg4)
            if si is not None:
                w0 = si * c_bl2.shard_slots // P
                w1 = w0 + c_bl2.shard_rows[si] // P
                emit_epilogue(cx, f"bl2s{si}", f0c_bl, f1c_bl, flocal, acc_sb,
                              w0, w1)
                shard_ag(c_bl2, acc_sb, bl2_agin, abl_tiles, si)

        emit_spmm(cx, "bl2", pp["bl2"], bl2_windows, g_in["bl2"]["idx"],
                  g_in["bl2"]["sel"], flocal, bl2_hook)

        # ---- phase 4: agg spmm + AG
        nsg5 = len(pp["ag"]["prog"])

        def ag_hook(g):
            si = last_sg_of_shard(c_ag, g, nsg5)
            if si is not None:
                shard_ag(c_ag, flocal, ag_agin, ilb_tiles, si)

        emit_spmm(cx, "ag", pp["ag"], ag_windows, g_in["ag"]["idx"],
                  g_in["ag"]["sel"], flocal, ag_hook)

        # ---- phase 5: loss
        assert len(abl_tiles) == 1 and len(ilb_tiles) == 1
        bsh = BATCH // NCORES
        ng = bsh // P
        lidx_sb = {}
        for k, t in lidx.items():
            s = cx.lp.tile([P, t.shape[1]], I32, tag=f"li_{k}")
            nc.sync.dma_start(out=s[:], in_=t[:])
            lidx_sb[k] = s
        for k, t in aug_in.items():
            s = cx.lp.tile([P, 16], I32, tag=f"li_{k}")
            nc.sync.dma_start(out=s[:], in_=t[:])
            lidx_sb[k] = s

        ail_ap = ail_tiles[0][:]
        abl_ap = abl_tiles[0][:]
        ilb_ap = ilb_tiles[0][:]

        def gather(tag, ap, idxk, ncols):
            sb = cx.lp.tile([P, ncols * D], F32, tag=tag)
            indirect_gather_rows(cx, sb, ap, lidx_sb[idxk], ncols)
            return sb

        pos_u_il = gather("pos_u_il", ail_ap, "u_il", ng)
        pos_u_bl = gather("pos_u_bl", abl_ap, "u_bl", ng)
        b_bl0 = gather("b_bl0", abl_ap, "b_bl0", ng)
        b_bl1 = gather("b_bl1", abl_ap, "b_bl1", ng)
        aug_u = gather("aug_u", abl_ap, "aug_u_bl", 16)
        aug_b = gather("aug_b", abl_ap, "aug_b0_bl", 16)

        # normalized copy of pos_u_il for c1 (BPR later needs the raw rows)
        pos_u_il_n = cx.lp.tile([P, ng * D], F32, tag="pos_u_il_n")
        nc.vector.tensor_copy(pos_u_il_n[:], pos_u_il[:, :ng * D])
        normalize_rows(cx, pos_u_il_n, ng, "npu")
        normalize_rows(cx, aug_u, 16, "nau")
        normalize_rows(cx, aug_b, 16, "nab")

        part = cx.lp.tile([P, 4], F32, tag="parts")
        nc.vector.memset(part[:], 0.0)

        def closs_partial(pos_my, aug_full, aug_my_cols, out_col):
            posT = transpose_groups(cx, pos_my, ng, "pT")
            augT = transpose_groups(cx, aug_full, 16, "aT")
            ps = cx.lp.tile([P, ng], F32, tag="ps")
            rowdot(cx, pos_my, aug_my_cols, ps, ng, f"psd{out_col}")
            lse = cx.lp.tile([P, ng], F32, tag="lse")
            for g in range(ng):
                ttl_ps = cx.psp.tile([P, 512], F32, space="PSUM", tag="ttl", bufs=1)
                ttl = cx.lp.tile([P, BATCH], F32, tag="ttl")
                for nb_ in range(BATCH // 512):
                    nc.tensor.matmul(
                        out=ttl_ps[:, :512],
                        lhsT=posT[:D, g * P:(g + 1) * P],
                        rhs=augT[:D, nb_ * 512:(nb_ + 1) * 512],
                        start=True, stop=True)
                    nc.vector.tensor_copy(ttl[:, nb_ * 512:(nb_ + 1) * 512], ttl_ps[:, :512])
                mx = cx.lp.tile([P, 1], F32, tag="mx")
                nc.vector.reduce_max(mx[:], ttl[:].rearrange("p (w d) -> p w d", w=1),
                                     axis=mybir.AxisListType.X)
                nmx = cx.lp.tile([P, 1], F32, tag="nmx")
                nc.vector.tensor_scalar_mul(nmx[:], mx[:], -4.0)
                ex = cx.lp.tile([P, BATCH], F32, tag="ex")
                se = cx.lp.tile([P, 1], F32, tag="se")
                nc.scalar.activation(ex[:], ttl[:], AF.Exp, bias=nmx[:, :1], scale=4.0,
                                     accum_out=se[:, :1])
                ln = cx.lp.tile([P, 1], F32, tag="ln")
                nc.scalar.activation(ln[:], se[:], AF.Ln)
                m4 = cx.lp.tile([P, 1], F32, tag="m4")
                nc.vector.tensor_scalar_mul(m4[:], mx[:], 4.0)
                nc.vector.tensor_add(lse[:, g:g + 1], ln[:], m4[:])
            t4 = cx.lp.tile([P, ng], F32, tag="t4")
            nc.vector.tensor_scalar_mul(t4[:], ps[:], 4.0)
            nc.vector.tensor_tensor(out=t4[:], in0=t4[:], in1=lse[:], op=ALU.subtract)
            nc.vector.reduce_sum(part[:, out_col:out_col + 1],
                                 t4[:].rearrange("p (w d) -> p w d", w=1),
                                 axis=mybir.AxisListType.X)

        aug_u_my = gather("aug_u_my", abl_ap, "u_bl", ng)
        normalize_rows(cx, aug_u_my, ng, "naum")
        aug_b_my = gather("aug_b_my", abl_ap, "b_bl0", ng)
        normalize_rows(cx, aug_b_my, ng, "nabm")
        closs_partial(pos_u_il_n, aug_u, aug_u_my, 1)

        b_il0 = gather("b_il0", ilb_ap, "b_il0", ng)
        b_il1 = gather("b_il1", ilb_ap, "b_il1", ng)
        pr0 = cx.lp.tile([P, ng], F32, tag="pr0")
        pr1 = cx.lp.tile([P, ng], F32, tag="pr1")
        tmp = cx.lp.tile([P, ng], F32, tag="prt")
        rowdot(cx, pos_u_il, b_il0, pr0, ng, "d0")
        rowdot(cx, pos_u_bl, b_bl0, tmp, ng, "d1")
        nc.vector.tensor_add(pr0[:], pr0[:], tmp[:])
        rowdot(cx, pos_u_il, b_il1, pr1, ng, "d2")
        rowdot(cx, pos_u_bl, b_bl1, tmp, ng, "d3")
        nc.vector.tensor_add(pr1[:], pr1[:], tmp[:])
        x = cx.lp.tile([P, ng], F32, tag="bprx")
        nc.vector.tensor_tensor(out=x[:], in0=pr1[:], in1=pr0[:], op=ALU.subtract)
        negx = cx.lp.tile([P, ng], F32, tag="bprnx")
        nc.vector.tensor_scalar_mul(negx[:], x[:], -1.0)
        nax = cx.lp.tile([P, ng], F32, tag="bprax")
        nc.vector.tensor_tensor(out=nax[:], in0=x[:], in1=negx[:], op=ALU.min)
        e = cx.lp.tile([P, ng], F32, tag="bpre")
        nc.scalar.activation(e[:], nax[:], AF.Exp)
        nc.vector.tensor_scalar_add(e[:], e[:], 1.0)
        l1p = cx.lp.tile([P, ng], F32, tag="bprl")
        nc.scalar.activation(l1p[:], e[:], AF.Ln)
        sp = cx.lp.tile([P, ng], F32, tag="bprsp")
        nc.vector.tensor_scalar_max(sp[:], x[:], 0.0)
        nc.vector.tensor_add(sp[:], sp[:], l1p[:])
        nc.vector.reduce_sum(part[:, 0:1], sp[:].rearrange("p (w d) -> p w d", w=1),
                             axis=mybir.AxisListType.X)

        my_pos_b = cx.lp.tile([P, ng * D], F32, tag="my_pb")
        nc.vector.tensor_copy(my_pos_b[:], b_il0[:, :ng * D])
        normalize_rows(cx, my_pos_b, ng, "npb")
        closs_partial(my_pos_b, aug_b, aug_b_my, 2)

        pp_ps = cx.psp.tile([P, 512], F32, space="PSUM", tag="ppps", bufs=1)
        nc.tensor.matmul(out=pp_ps[:1, :4], lhsT=ones_col[:], rhs=part[:],
                         start=True, stop=True)
        psum_sb = cx.lp.tile([1, 4], F32, tag="psums")
        nc.vector.tensor_copy(psum_sb[:], pp_ps[:1, :4])
        ar_in = cx.dramp.tile([1, 4], F32, tag="ar_in")
        ar_out = cx.dramp.tile([1, 4], F32, addr_space="Shared", tag="ar_out")
        nc.sync.dma_start(out=ar_in[:], in_=psum_sb[:])
        nc.gpsimd.collective_compute(
            "AllReduce", ALU.add, replica_groups=[list(range(NCORES))],
            ins=[ar_in[:].opt()], outs=[ar_out[:].opt()])
        fin = cx.lp.tile([1, 4], F32, tag="fin")
        nc.sync.dma_start(out=fin[:], in_=ar_out[:])
        res = cx.lp.tile([1, 2], F32, tag="res")
        nc.vector.tensor_scalar_mul(res[:, 0:1], fin[:, 0:1], 1.0 / BATCH)
        t = cx.lp.tile([1, 1], F32, tag="rt")
        nc.vector.tensor_add(t[:], fin[:, 1:2], fin[:, 2:3])
        nc.vector.tensor_scalar_mul(res[:, 1:2], t[:], -0.5 / BATCH)
        nc.sync.dma_start(out=out_t[:], in_=res[:])

        for p in reversed(es):
            p.__exit__(None, None, None)
    nc.compile()
    return nc


# ---------------------------------------------------------------- entry point

def _install_ntff_hook():
    if "antenv.axon_hooks" in sys.modules:
        return
    try:
        mod = types.ModuleType("antenv.axon_hooks")
        _hook = [None]
        mod.set_axon_ntff_profile_hook = lambda h: _hook.__setitem__(0, h)
        mod.get_axon_ntff_profile_hook = lambda: _hook[0]
        sys.modules["antenv.axon_hooks"] = mod
        import antenv
        antenv.axon_hooks = mod
        from trn_agent_boot.trn_boot import _ntff_profile_via_ctypes
        hook = _ntff_profile_via_ctypes("/opt/axon/libaxon_pjrt.so")
        if hook is not None:
            mod.set_axon_ntff_profile_hook(hook)
    except Exception:
        pass


def make_in_maps(pp):
    maps = []
    for c in range(NCORES):
        m = {
            "f0_il": pp["f0_il"], "f0_bl": pp["f0_bl"],
            "f0c_il": pp["f0c_il"][c], "f0c_bl": pp["f0c_bl"][c],
            "aug_u_bl": pp["aug_u_bl"], "aug_b0_bl": pp["aug_b0_bl"],
        }
        for nm in ("il1", "bl1", "il2", "bl2", "ag"):
            m[f"{nm}_idx"] = pp[nm]["idx"][c]
            m[f"{nm}_sel"] = pp[nm]["sel"][c]
        for k, v in pp["loss"][c].items():
            m[f"loss_{k}"] = v
        maps.append(m)
    return maps


_CACHE = {}


def kernel(**inputs) -> np.ndarray:
    _install_ntff_hook()
    pp = preprocess(inputs)
    key = "full"
    if key not in _CACHE:
        _CACHE[key] = build(pp)
    nc = _CACHE[key]
    in_maps = make_in_maps(pp)
    trace = bool(int(os.environ.get("DSCBR_TRACE", "0")))
    res = run_bass_kernel_spmd(nc, in_maps, core_ids=list(range(NCORES)), trace=trace)
    if trace and res.exec_time_ns:
        print(f"HW exec time: {res.exec_time_ns} ns")
    out = res.results[0]["out"].reshape(2).astype(np.float32)
    return out
